# revision 1
# baseline (speedup 1.0000x reference)
"""GAdapter (GNN message passing + adapter MLP) Bass kernel for Trainium2, 8-core SPMD.

Entry point: kernel(**inputs) -> np.ndarray [1, N, H] float32.

Two-launch design (no collectives).

k1 (per core): LN of own x slab -> eta (residual, f32 out) and
    hM = eta @ (diag(pre_g) M) + pre_b M   (bf16 out)      M = down_w^T up_w^T
Host: concat hM slabs -> full table [N+1, H] bf16 (last row zero sentinel),
    replicate to all cores.
k2 (per core): per 128-row tile, gather hM[dst] for group-packed edge slots,
    build one-hot lhsT per GROUP (G batches share one scalar column: partition k
    is pinned to source row R[k] for the whole group), PE-accumulate into PSUM
    => z2 tile; relu + residual + post-LN; DMA out.
"""

from contextlib import ExitStack
from dataclasses import dataclass

import numpy as np

import concourse.bass as bass
import concourse.tile as tile
from concourse import bacc, mybir

F32 = mybir.dt.float32
BF16 = mybir.dt.bfloat16
I16 = mybir.dt.int16
EPS = 1e-5


@dataclass
class Cfg:
    N: int = 16384
    H: int = 128
    NCORES: int = 8
    G: int = 4        # batches per one-hot group
    NG: int = 9       # groups per 128-row tile (data-dependent max)
    CHUNK: int = 1    # row-tiles per dma_gather call
    use_bM: bool = False
    use_c: bool = False
    use_pre_gb: bool = False
    use_post_gb: bool = False
    oh_bufs: int = 12
    gath_bufs: int = 8
    reps: int = 1

    @property
    def SLAB(self):
        return self.N // self.NCORES

    @property
    def T(self):
        return self.SLAB // 128

    @property
    def NB(self):
        return self.NG * self.G  # batches per tile


def build_k1(cfg: Cfg):
    nc = bacc.Bacc("TRN2", target_bir_lowering=False, debug=False, num_devices=cfg.NCORES)
    H, T = cfg.H, cfg.T
    x_slab = nc.dram_tensor("x_slab", [cfg.SLAB, H], F32, kind="ExternalInput")
    ident_in = nc.dram_tensor("ident", [128, 128], F32, kind="ExternalInput")
    down_w_in = nc.dram_tensor("down_w", [32, H], F32, kind="ExternalInput")
    up_wT_in = nc.dram_tensor("up_wT", [32, H], F32, kind="ExternalInput")
    pre_g_in = nc.dram_tensor("pre_g", [H, 1], F32, kind="ExternalInput")
    pre_b_in = nc.dram_tensor("pre_b", [H, 1], F32, kind="ExternalInput")
    hM_out = nc.dram_tensor("hM", [cfg.SLAB, H], BF16, kind="ExternalOutput")
    eta_out = nc.dram_tensor("eta", [cfg.SLAB, H], F32, kind="ExternalOutput")

    with tile.TileContext(nc) as tc, ExitStack() as ctx:
        const = ctx.enter_context(tc.tile_pool(name="const", bufs=1))
        xin = ctx.enter_context(tc.tile_pool(name="xin", bufs=3))
        stat = ctx.enter_context(tc.tile_pool(name="stat", bufs=4))
        work = ctx.enter_context(tc.tile_pool(name="work", bufs=3))
        psA = ctx.enter_context(tc.tile_pool(name="psA", bufs=3, space="PSUM"))
        psP = ctx.enter_context(tc.tile_pool(name="psP", bufs=2, space="PSUM"))

        ident_t = const.tile([128, 128], F32)
        nc.sync.dma_start(ident_t[:], ident_in[:])
        down_w_t = const.tile([32, H], F32)
        nc.sync.dma_start(down_w_t[:], down_w_in[:])
        up_wT_t = const.tile([32, H], F32)
        nc.sync.dma_start(up_wT_t[:], up_wT_in[:])
        pre_g_t = const.tile([H, 1], F32)
        nc.sync.dma_start(pre_g_t[:], pre_g_in[:])
        eps_t = const.tile([128, 1], F32)
        nc.vector.memset(eps_t[:], EPS)

        psM = psP.tile([128, H], F32, tag="pro")
        nc.tensor.matmul(psM[:], down_w_t[:], up_wT_t[:], start=True, stop=True)
        Mg_t = const.tile([128, H], F32)
        nc.vector.tensor_scalar(Mg_t[:], psM[:], pre_g_t[:], None, mybir.AluOpType.mult)

        if cfg.use_bM or cfg.use_pre_gb:
            ones_1 = const.tile([1, 128], F32)
            nc.vector.memset(ones_1[:], 1.0)
        if cfg.use_bM:
            M_t = const.tile([128, H], F32)
            nc.scalar.activation(M_t[:], psM[:], mybir.ActivationFunctionType.Copy)
            pre_b_t = const.tile([H, 1], F32)
            nc.sync.dma_start(pre_b_t[:], pre_b_in[:])
            ps_bM = psP.tile([1, H], F32, tag="pro")
            nc.tensor.matmul(ps_bM[:], pre_b_t[:], M_t[:], start=True, stop=True)
            bM_t = const.tile([1, H], F32)
            nc.scalar.activation(bM_t[:], ps_bM[:], mybir.ActivationFunctionType.Copy)
        if cfg.use_pre_gb:
            # broadcast tiles of pre_g / pre_b along partitions (for residual h)
            pre_g_r = const.tile([1, H], F32)
            nc.sync.dma_start(pre_g_r[:], pre_g_in.ap().rearrange("h one -> one h"))
            pre_b_r = const.tile([1, H], F32)
            nc.sync.dma_start(pre_b_r[:], pre_b_in.ap().rearrange("h one -> one h"))
            ps_g = psP.tile([128, H], F32, tag="pro")
            nc.tensor.matmul(ps_g[:], ones_1[:], pre_g_r[:], start=True, stop=True)
            gb_t = const.tile([128, H], F32)
            nc.scalar.activation(gb_t[:], ps_g[:], mybir.ActivationFunctionType.Copy)
            ps_b = psP.tile([128, H], F32, tag="pro")
            nc.tensor.matmul(ps_b[:], ones_1[:], pre_b_r[:], start=True, stop=True)
            bb_t = const.tile([128, H], F32)
            nc.scalar.activation(bb_t[:], ps_b[:], mybir.ActivationFunctionType.Copy)

        for _rep in range(cfg.reps):
            x_all = xin.tile([128, T, H], F32, tag="xall")
            x_src = x_slab.ap().rearrange("(t p) h -> p t h", p=128)
            QT = max(1, T // 4)
            for q0 in range(0, T, QT):
                q1 = min(T, q0 + QT)
                nc.sync.dma_start(x_all[:, q0:q1, :], x_src[:, q0:q1, :])
            eta_all = xin.tile([128, T, H], F32, tag="etaall")
            hM_all = xin.tile([128, T, H], BF16, tag="hMall")
            for t in range(T):
                xt = x_all[:, t, :]
                st6 = stat.tile([128, 6], F32, tag="st6")
                nc.vector.bn_stats(st6[:], xt[:])
                mv = stat.tile([128, 2], F32, tag="mv")
                nc.vector.bn_aggr(mv[:], st6[:])
                sd = stat.tile([128, 1], F32, tag="sd")
                nc.scalar.activation(sd[:], mv[:, 1:2], mybir.ActivationFunctionType.Sqrt, bias=eps_t[:])
                rstd = stat.tile([128, 1], F32, tag="rstd")
                nc.vector.reciprocal(rstd[:], sd[:])
                eta = eta_all[:, t, :]
                nc.vector.tensor_scalar(
                    eta, xt[:], mv[:, 0:1], rstd[:],
                    mybir.AluOpType.subtract, mybir.AluOpType.mult,
                )
                psT = psA.tile([128, H], F32, tag="psT")
                nc.tensor.transpose(psT[:], eta, ident_t[:])
                etaT = work.tile([128, H], F32, tag="etaT")
                if t % 2 == 0:
                    nc.scalar.activation(etaT[:], psT[:], mybir.ActivationFunctionType.Copy)
                else:
                    nc.vector.tensor_copy(etaT[:], psT[:])
                pshM = psA.tile([128, H], F32, tag="pshM")
                nc.tensor.matmul(pshM[:], etaT[:], Mg_t[:], start=True, stop=not cfg.use_bM)
                if cfg.use_bM:
                    nc.tensor.matmul(pshM[:], ones_1[:], bM_t[:], start=False, stop=True)
                nc.scalar.activation(hM_all[:, t, :], pshM[:], mybir.ActivationFunctionType.Copy)
                if cfg.use_pre_gb:
                    nc.vector.tensor_tensor(eta, eta, gb_t[:], mybir.AluOpType.mult)
                    nc.vector.tensor_tensor(eta, eta, bb_t[:], mybir.AluOpType.add)
            eta_dst = eta_out.ap().rearrange("(t p) h -> p t h", p=128)
            hM_dst = hM_out.ap().rearrange("(t p) h -> p t h", p=128)
            HT = T // 2
            nc.sync.dma_start(eta_dst[:, 0:HT, :], eta_all[:, 0:HT, :])
            nc.sync.dma_start(eta_dst[:, HT:T, :], eta_all[:, HT:T, :])
            nc.scalar.dma_start(hM_dst[:, 0:HT, :], hM_all[:, 0:HT, :])
            nc.scalar.dma_start(hM_dst[:, HT:T, :], hM_all[:, HT:T, :])

    nc.compile()
    return nc


def build_k2(cfg: Cfg):
    nc = bacc.Bacc("TRN2", target_bir_lowering=False, debug=False, num_devices=cfg.NCORES)
    H, T, NG, G, CH = cfg.H, cfg.T, cfg.NG, cfg.G, cfg.CHUNK
    NB = cfg.NB
    # chunk sizes: CH-sized except last two are single tiles (shorter tail)
    chunks = []
    rem = T
    while rem > 0:
        if rem <= 2:
            chunks.append(1)
            rem -= 1
        else:
            c = min(CH, rem - 2)
            chunks.append(c)
            rem -= c
    chunk_starts = [sum(chunks[:i]) for i in range(len(chunks))]
    n_chunks = len(chunks)
    S_tile = NB * 8  # idx columns per row-tile

    table_in = nc.dram_tensor("table", [cfg.N + 1, H], BF16, kind="ExternalInput")
    eta_in = nc.dram_tensor("eta", [cfg.SLAB, H], F32, kind="ExternalInput")
    idx_plane = nc.dram_tensor("idx_plane", [128, T * S_tile], I16, kind="ExternalInput")
    R_plane = nc.dram_tensor("R_plane", [128, T * NG], F32, kind="ExternalInput")
    iota_in = nc.dram_tensor("iota", [128, G * 128], BF16, kind="ExternalInput")
    down_b_in = nc.dram_tensor("down_b", [32, 1], F32, kind="ExternalInput")
    up_wT_in = nc.dram_tensor("up_wT", [32, H], F32, kind="ExternalInput")
    up_b_in = nc.dram_tensor("up_b", [1, H], F32, kind="ExternalInput")
    post_g_in = nc.dram_tensor("post_g", [1, H], F32, kind="ExternalInput")
    post_b_in = nc.dram_tensor("post_b", [1, H], F32, kind="ExternalInput")
    y_out = nc.dram_tensor("y", [cfg.SLAB, H], F32, kind="ExternalOutput")

    with tile.TileContext(nc) as tc, ExitStack() as ctx:
        const = ctx.enter_context(tc.tile_pool(name="const", bufs=1))
        etap = ctx.enter_context(tc.tile_pool(name="etap", bufs=3))
        stat = ctx.enter_context(tc.tile_pool(name="stat", bufs=4))
        ohp = ctx.enter_context(tc.tile_pool(name="oh", bufs=cfg.oh_bufs))
        gathp = ctx.enter_context(tc.tile_pool(name="gath", bufs=cfg.gath_bufs))
        outp = ctx.enter_context(tc.tile_pool(name="outp", bufs=3))
        psZ = ctx.enter_context(tc.tile_pool(name="psZ", bufs=4, space="PSUM"))
        psP = ctx.enter_context(tc.tile_pool(name="psP", bufs=2, space="PSUM"))

        iota_t = const.tile([128, G * 128], BF16)
        nc.sync.dma_start(iota_t[:], iota_in[:])
        idxp_t = const.tile([128, T * S_tile], I16)
        for cc in range(n_chunks):
            c0 = chunk_starts[cc] * S_tile
            c1 = c0 + chunks[cc] * S_tile
            nc.sync.dma_start(idxp_t[:, c0:c1], idx_plane[:, c0:c1])
        Rp_t = const.tile([128, T * NG], F32)
        nc.sync.dma_start(Rp_t[:], R_plane[:])

        if cfg.use_c or cfg.use_post_gb:
            ones_1 = const.tile([1, 128], F32)
            nc.vector.memset(ones_1[:], 1.0)
        if cfg.use_c:
            down_b_t = const.tile([32, 1], F32)
            nc.sync.dma_start(down_b_t[:], down_b_in[:])
            up_wT_t = const.tile([32, H], F32)
            nc.sync.dma_start(up_wT_t[:], up_wT_in[:])
            up_b_t = const.tile([1, H], F32)
            nc.sync.dma_start(up_b_t[:], up_b_in[:])
            ps_c = psP.tile([1, H], F32, tag="pro")
            nc.tensor.matmul(ps_c[:], down_b_t[:], up_wT_t[:], start=True, stop=True)
            c_t = const.tile([1, H], F32)
            nc.vector.tensor_tensor(c_t[:], ps_c[:], up_b_t[:], mybir.AluOpType.add)
        if cfg.use_post_gb:
            post_g_t = const.tile([1, H], F32)
            nc.sync.dma_start(post_g_t[:], post_g_in[:])
            post_b_t = const.tile([1, H], F32)
            nc.sync.dma_start(post_b_t[:], post_b_in[:])
            ps_g2 = psP.tile([128, H], F32, tag="pro")
            nc.tensor.matmul(ps_g2[:], ones_1[:], post_g_t[:], start=True, stop=True)
            postg_b_t = const.tile([128, H], F32)
            nc.scalar.activation(postg_b_t[:], ps_g2[:], mybir.ActivationFunctionType.Copy)
            ps_b2 = psP.tile([128, H], F32, tag="pro")
            nc.tensor.matmul(ps_b2[:], ones_1[:], post_b_t[:], start=True, stop=True)
            postb_b_t = const.tile([128, H], F32)
            nc.scalar.activation(postb_b_t[:], ps_b2[:], mybir.ActivationFunctionType.Copy)

        eps_t = const.tile([128, 1], F32)
        nc.vector.memset(eps_t[:], EPS)

        for _rep in range(cfg.reps):
            eta_all = const.tile([128, T, H], F32)
            nc.sync.dma_start(eta_all[:], eta_in.ap().rearrange("(t p) h -> p t h", p=128))

            for cc in range(n_chunks):
                CHc = chunks[cc]
                t_base = chunk_starts[cc]
                gath = gathp.tile([128, CH * NB, H], BF16, tag="gath")
                s0 = t_base * S_tile
                nc.gpsimd.dma_gather(
                    out_ap=gath[:, 0 : CHc * NB, :],
                    in_ap=table_in[:],
                    idxs_ap=idxp_t[:, s0 : s0 + CHc * S_tile],
                    num_idxs=CHc * NB * 128,
                    num_idxs_reg=CHc * NB * 128,
                    elem_size=H,
                    single_packet=False,
                )
                for ti in range(CHc):
                    t = t_base + ti
                    psz = psZ.tile([128, H], F32)
                    for g in range(NG):
                        oh = ohp.tile([128, G * 128], BF16, tag="oh")
                        nc.vector.tensor_scalar(
                            oh[:],
                            iota_t[:],
                            Rp_t[:, t * NG + g : t * NG + g + 1],
                            None,
                            mybir.AluOpType.is_equal,
                        )
                        for j in range(G):
                            b = g * G + j
                            nc.tensor.matmul(
                                psz[:],
                                oh[:, j * 128 : (j + 1) * 128],
                                gath[:, ti * NB + b, :],
                                start=(b == 0),
                                stop=(b == NB - 1 and not cfg.use_c),
                            )
                    if cfg.use_c:
                        nc.tensor.matmul(psz[:], ones_1[:], c_t[:], start=False, stop=True)
                    # epilogue
                    v = outp.tile([128, H], F32, tag="v")
                    nc.scalar.activation(v[:], psz[:], mybir.ActivationFunctionType.Relu)
                    v2 = outp.tile([128, H], F32, tag="v2")
                    nc.vector.tensor_tensor(v2[:], v[:], eta_all[:, t, :], mybir.AluOpType.add)
                    st6b = stat.tile([128, 6], F32, tag="st6b")
                    nc.vector.bn_stats(st6b[:], v2[:])
                    mvb = stat.tile([128, 2], F32, tag="mvb")
                    nc.vector.bn_aggr(mvb[:], st6b[:])
                    sdb = stat.tile([128, 1], F32, tag="sdb")
                    nc.scalar.activation(sdb[:], mvb[:, 1:2], mybir.ActivationFunctionType.Sqrt, bias=eps_t[:])
                    rstdb = stat.tile([128, 1], F32, tag="rstdb")
                    nc.vector.reciprocal(rstdb[:], sdb[:])
                    yt = outp.tile([128, H], F32, tag="yt")
                    nc.vector.tensor_scalar(
                        yt[:], v2[:], mvb[:, 0:1], rstdb[:],
                        mybir.AluOpType.subtract, mybir.AluOpType.mult,
                    )
                    if cfg.use_post_gb:
                        nc.vector.tensor_tensor(yt[:], yt[:], postg_b_t[:], mybir.AluOpType.mult)
                        nc.vector.tensor_tensor(yt[:], yt[:], postb_b_t[:], mybir.AluOpType.add)
                    nc.sync.dma_start(y_out[t * 128 : (t + 1) * 128, :], yt[:])

    nc.compile()
    return nc


# ---------------------------------------------------------------------------
# host-side prep
# ---------------------------------------------------------------------------


def pack_edges(src_s, dst_s, n_tiles, G, N):
    """Group-pack sorted edges. Returns (claim_R [n_tiles, list], claim_dst).

    For each 128-row tile: rows' edges split into claims of <= G edges; claim i
    -> (group i//128, partition i%128). Returns per-tile arrays:
      R[t]   : [n_claims_t]  source row offset (0..127) per claim
      DST[t] : [n_claims_t, G] dst indices (sentinel N where empty)
    """
    Rs, DSTs = [], []
    tile_of = src_s >> 7
    bounds = np.searchsorted(tile_of, np.arange(n_tiles + 1))
    for t in range(n_tiles):
        a, b = bounds[t], bounds[t + 1]
        rr = (src_s[a:b] & 127).astype(np.int64)
        dd = dst_s[a:b]
        # edges sorted by src -> rr sorted; split runs into <=G chunks
        R_list = []
        D_list = []
        start = 0
        n = b - a
        while start < n:
            r = rr[start]
            end = start
            while end < n and rr[end] == r and end - start < G:
                end += 1
            d = np.full(G, N, dtype=np.int64)
            d[: end - start] = dd[start:end]
            R_list.append(r)
            D_list.append(d)
            start = end
        Rs.append(np.array(R_list, dtype=np.int64))
        DSTs.append(np.array(D_list, dtype=np.int64).reshape(-1, G))
    return Rs, DSTs


def prep_inputs(x, edge_index, down_w, down_b, up_w, up_b, pre_g, pre_b, post_g,
                post_b, cfg=None):
    N = x.shape[1]
    H = x.shape[2]
    src = np.asarray(edge_index[0], dtype=np.int64)
    dst = np.asarray(edge_index[1], dtype=np.int64)
    order = np.argsort(src, kind="stable")
    src_s = src[order]
    dst_s = dst[order]
    n_tiles = N // 128

    if cfg is None:
        cfg = Cfg(N=N, H=H)
    G = cfg.G
    Rs, DSTs = pack_edges(src_s, dst_s, n_tiles, G, N)
    NG = max(1, int(np.ceil(max(len(r) for r in Rs) / 128)))
    cfg.NG = NG
    cfg.use_bM = bool(np.any(pre_b != 0))
    cfg.use_c = bool(np.any(down_b != 0) or np.any(up_b != 0))
    cfg.use_pre_gb = bool(np.any(pre_g != 1) or np.any(pre_b != 0))
    cfg.use_post_gb = bool(np.any(post_g != 1) or np.any(post_b != 0))
    T = cfg.T
    NB = cfg.NB

    import ml_dtypes

    iota = np.tile(np.arange(128, dtype=np.float32), (128, cfg.G)).astype(ml_dtypes.bfloat16)
    ident = np.eye(128, dtype=np.float32)
    wT = np.ascontiguousarray(np.asarray(up_w, np.float32).T)

    k1_maps, k2_maps = [], []
    for c in range(cfg.NCORES):
        t0 = c * T
        # per-tile slot arrays: dst_slot [T, NG*128, G], R_slot [T, 128, NG]
        idx_cols = []
        Rp = np.zeros((128, T * NG), np.float32)
        dst_all = np.full((T, NG, G, 128), N, dtype=np.int64)  # [t, g, j, k]
        for tt in range(T):
            R_t = Rs[t0 + tt]
            D_t = DSTs[t0 + tt]  # [n_claims, G]
            nclaims = len(R_t)
            ggrid = np.arange(nclaims) // 128
            kgrid = np.arange(nclaims) % 128
            dst_all[tt, ggrid, :, kgrid] = D_t  # [n_claims, G] -> (g, :, k)
            Rp[kgrid, tt * NG + ggrid] = R_t
        # gather idx order: chunk cc covers tiles [cc*CH, cc*CH+CH);
        # within: tile-major, batch b = g*G+j, partition k: idx[(b*128)+k]
        CH = cfg.CHUNK
        flat_tile = dst_all.transpose(0, 1, 2, 3).reshape(T, NB * 128)  # [t, b*128+k]
        for cc in range(T // CH):
            fl = flat_tile[cc * CH : (cc + 1) * CH].reshape(-1)
            w = fl.reshape(-1, 16).T
            idx_cols.append(np.tile(w, (8, 1)))
        idx_plane = np.concatenate(idx_cols, axis=1).astype(np.int16)

        k1_maps.append({
            "x_slab": np.ascontiguousarray(x[0, c * cfg.SLAB : (c + 1) * cfg.SLAB, :], dtype=np.float32),
            "ident": ident,
            "down_w": np.asarray(down_w, np.float32),
            "up_wT": wT,
            "pre_g": np.asarray(pre_g, np.float32).reshape(H, 1),
            "pre_b": np.asarray(pre_b, np.float32).reshape(H, 1),
        })
        k2_maps.append({
            "idx_plane": np.ascontiguousarray(idx_plane),
            "R_plane": np.ascontiguousarray(Rp),
            "iota": iota,
            "down_b": np.asarray(down_b, np.float32).reshape(-1, 1),
            "up_wT": wT,
            "up_b": np.asarray(up_b, np.float32).reshape(1, H),
            "post_g": np.asarray(post_g, np.float32).reshape(1, H),
            "post_b": np.asarray(post_b, np.float32).reshape(1, H),
        })
    return cfg, k1_maps, k2_maps


def run_full(inputs, cfg=None, runner=None):
    """Complete two-launch execution. runner(nc, in_maps) -> list of out dicts."""
    import ml_dtypes
    from concourse.bass_utils import run_bass_kernel_spmd

    if runner is None:
        def runner(nc, in_maps):
            res = run_bass_kernel_spmd(nc, in_maps, list(range(8)))
            return res.results

    cfg, k1_maps, k2_maps = prep_inputs(**inputs, cfg=cfg)
    nc1 = build_k1(cfg)
    r1 = runner(nc1, k1_maps)
    table = np.concatenate([r1[c]["hM"] for c in range(cfg.NCORES)], axis=0)
    table = np.concatenate([table, np.zeros((1, cfg.H), table.dtype)], axis=0)
    for c in range(cfg.NCORES):
        k2_maps[c]["table"] = table
        k2_maps[c]["eta"] = r1[c]["eta"]
    nc2 = build_k2(cfg)
    r2 = runner(nc2, k2_maps)
    y = np.concatenate([r2[c]["y"] for c in range(cfg.NCORES)], axis=0)
    return y[None]




# ---------------------------------------------------------------------------
# main entry
# ---------------------------------------------------------------------------

_CACHE = {}


def _run_spmd(nc, maps, cores):
    from concourse.bass_utils import run_bass_kernel_spmd

    last_err = None
    for _attempt in range(3):
        try:
            return run_bass_kernel_spmd(nc, maps, cores).results
        except Exception as e:  # transient device/transport errors
            last_err = e
            import time as _time
            _time.sleep(2.0)
    raise last_err


def kernel(x, edge_index, down_w, down_b, up_w, up_b, pre_g, pre_b, post_g, post_b):
    import numpy as _np

    inputs = dict(x=_np.asarray(x), edge_index=_np.asarray(edge_index),
                  down_w=_np.asarray(down_w), down_b=_np.asarray(down_b),
                  up_w=_np.asarray(up_w), up_b=_np.asarray(up_b),
                  pre_g=_np.asarray(pre_g), pre_b=_np.asarray(pre_b),
                  post_g=_np.asarray(post_g), post_b=_np.asarray(post_b))
    cfg, k1_maps, k2_maps = prep_inputs(**inputs)
    key = (cfg.N, cfg.H, cfg.G, cfg.NG, cfg.CHUNK, cfg.use_bM, cfg.use_c,
           cfg.use_pre_gb, cfg.use_post_gb)
    if key not in _CACHE:
        _CACHE[key] = (build_k1(cfg), build_k2(cfg))
    nc1, nc2 = _CACHE[key]
    cores = list(range(cfg.NCORES))
    r1 = _run_spmd(nc1, k1_maps, cores)
    table = _np.concatenate([r1[c]["hM"] for c in range(cfg.NCORES)], axis=0)
    table = _np.concatenate([table, _np.zeros((1, cfg.H), table.dtype)], axis=0)
    for c in range(cfg.NCORES):
        k2_maps[c]["table"] = table
        k2_maps[c]["eta"] = r1[c]["eta"]
    r2 = _run_spmd(nc2, k2_maps, cores)
    y = _np.concatenate([r2[c]["y"] for c in range(cfg.NCORES)], axis=0)
    return y[None].astype(_np.float32)



# revision 24
# speedup vs baseline: 1.8035x; 1.8035x over previous
"""GAdapter (GNN message passing + adapter MLP) Bass kernel for Trainium2, 8-core SPMD.

Entry point: kernel(**inputs) -> np.ndarray [1, N, H] float32.

Two-launch design (no collectives), aggregation in the down-projected space.

k1 (per core): LN of own x slab -> eta (residual, f16) and
    d = eta @ (diag(pre_g) down_w^T) [+ pre_b down_w^T]   [SLAB, 32] f32
Host: concat d slabs -> table [N+1, 64] f32 (cols 32:64 zero, last row zero
    sentinel), viewed as int64 [N+1, 32]; replicate to all cores.
k2 (per core): per 128-row tile, gather table rows (256B each = one idx per
    edge; int64 typing keeps the gather's free-size small) for group-packed
    edge slots. Per claim group g one f16 one-hot (stationary lhsT) times the
    f16 payload view (moving, strided slice [slots, 4, 0:32] skips the pad)
    accumulates psum[m, je*32+c]. Then one PE transpose and ONE K=128
    up-projection matmul against up_w^T stacked 4x gives z2 = agg @ up_w^T
    (the sum over edge-positions je happens inside the contraction);
    relu + residual + post-LN; DMA out.

Claim packing: a claim is <= G=4 edges sharing one source row within the
tile; claim i of a tile sits at slot (g=i//128, k=i%128); its 4 edges'
gathers land at out[k, g*4+je].
"""

from contextlib import ExitStack
from dataclasses import dataclass, field

import numpy as np

import concourse.bass as bass
import concourse.tile as tile
from concourse import bacc, mybir

F32 = mybir.dt.float32
F32R = mybir.dt.float32r
F16 = mybir.dt.float16
I64 = mybir.dt.int64
I32 = mybir.dt.int32
I16 = mybir.dt.int16
EPS = 1e-5


def _raw_dma_gather(g, out_ap, in_ap, idxs_ap, num_idxs, num_idxs_reg, elem_size,
                    elem_step, single_packet=False, queue_num=0):
    """dma_gather for sub-256B elements (elem read < 256B row stride).

    Same lowering as bass's dma_gather non-transpose DRAM path, minus the
    elem_size%256 assert (which only the transpose mode needs). The row
    stride (elem_step * dtype) must still be a 256B multiple.
    """
    from concourse.bass import exact_div

    stride_bytes = elem_step * mybir.dt.size(in_ap.dtype)
    stride_bytes_256 = exact_div(stride_bytes, 256)
    _in_ap = g.lower_ap_dma(in_ap, for_custom_bir_dma=True)
    _idxs_ap = g.lower_ap(idxs_ap)
    _out_ap = g.lower_ap(out_ap)
    return g.add_instruction(
        mybir.InstDMAGatherAnt(
            name=g.bass.get_next_instruction_name(),
            ins=[*_in_ap, _idxs_ap, g.lower_val_access(g.to_reg(num_idxs_reg))],
            outs=[_out_ap], transpose=False, num_idxs=num_idxs,
            elem_size=elem_size, stride_bytes_256=stride_bytes_256, gen_mode=0,
            single_packet=single_packet, queue_num=queue_num,
            sbuf_tokens_per_rank=0, sbuf_free_dim_per_rank=0,
            sbuf_free_dim_pad_per_rank=0, sbuf_byte_offset=0,
        ))


@dataclass
class Cfg:
    N: int = 16384
    H: int = 128
    B: int = 32
    NCORES: int = 8
    G: int = 4
    ngs: tuple = field(default_factory=lambda: tuple([9] * 16))  # per-tile claim groups
    use_bM: bool = False       # pre_b != 0 (bias into d)
    use_pre_gb: bool = False   # pre_g/pre_b non-identity (residual adjust)
    use_c: bool = False        # down_b/up_b != 0
    use_post_gb: bool = False  # post_g/post_b non-identity
    reps: int = 1

    @property
    def SLAB(self):
        return self.N // self.NCORES

    @property
    def T(self):
        return self.SLAB // 128


def build_k1(cfg: Cfg):
    nc = bacc.Bacc("TRN2", target_bir_lowering=False, debug=False, num_devices=cfg.NCORES)
    H, B, T = cfg.H, cfg.B, cfg.T
    x_slab = nc.dram_tensor("x_slab", [cfg.SLAB, H], F32, kind="ExternalInput")
    ident_in = nc.dram_tensor("ident", [128, 128], F16, kind="ExternalInput")
    dwT_in = nc.dram_tensor("dwT", [H, B], F16, kind="ExternalInput")
    c1_in = nc.dram_tensor("c1", [1, B], F16, kind="ExternalInput")
    pre_g_in = nc.dram_tensor("pre_g", [1, H], F16, kind="ExternalInput")
    pre_b_in = nc.dram_tensor("pre_b", [1, H], F16, kind="ExternalInput")
    d_out = nc.dram_tensor("d", [128, T * B], F16, kind="ExternalOutput")
    eta_out = nc.dram_tensor("eta", [128, T * H], F16, kind="ExternalOutput")

    with tile.TileContext(nc) as tc, ExitStack() as ctx:
        const = ctx.enter_context(tc.tile_pool(name="const", bufs=1))
        xin = ctx.enter_context(tc.tile_pool(name="xin", bufs=1))
        stat = ctx.enter_context(tc.tile_pool(name="stat", bufs=6))
        work = ctx.enter_context(tc.tile_pool(name="work", bufs=3))
        psT_p = ctx.enter_context(tc.tile_pool(name="psT", bufs=3, space="PSUM"))
        psD_p = ctx.enter_context(tc.tile_pool(name="psD", bufs=3, space="PSUM"))

        ident_t = const.tile([128, 128], F16)
        nc.sync.dma_start(ident_t[:], ident_in[:])
        dwT_t = const.tile([H, B], F16)
        nc.sync.dma_start(dwT_t[:], dwT_in[:])
        eps_t = const.tile([128, 1], F32)
        nc.vector.memset(eps_t[:], EPS)
        if cfg.use_bM:
            ones1 = const.tile([1, 128], F16)
            nc.vector.memset(ones1[:], 1.0)
            c1_t = const.tile([1, B], F16)
            nc.sync.dma_start(c1_t[:], c1_in[:])
        if cfg.use_pre_gb:
            pre_g_t = const.tile([1, H], F16)
            nc.sync.dma_start(pre_g_t[:], pre_g_in[:])
            pre_b_t = const.tile([1, H], F16)
            nc.sync.dma_start(pre_b_t[:], pre_b_in[:])
            ones_c = const.tile([1, 128], F16)
            nc.vector.memset(ones_c[:], 1.0)
            ps_g = psT_p.tile([128, H], F32, tag="pro", padded_shape=[128, 512])
            nc.tensor.matmul(ps_g[:], ones_c[:], pre_g_t[:], start=True, stop=True)
            gb_t = const.tile([128, H], F16)
            nc.scalar.activation(gb_t[:], ps_g[:], mybir.ActivationFunctionType.Copy)
            ps_b = psT_p.tile([128, H], F32, tag="pro", padded_shape=[128, 512])
            nc.tensor.matmul(ps_b[:], ones_c[:], pre_b_t[:], start=True, stop=True)
            bb_t = const.tile([128, H], F16)
            nc.scalar.activation(bb_t[:], ps_b[:], mybir.ActivationFunctionType.Copy)

        for _rep in range(cfg.reps):
            x_all = xin.tile([128, T, H], F32, tag="xall")
            x_src = x_slab.ap().rearrange("(t p) h -> p t h", p=128)
            HT = T // 2
            nc.sync.dma_start(x_all[:, 0:HT, :], x_src[:, 0:HT, :])
            nc.scalar.dma_start(x_all[:, HT:T, :], x_src[:, HT:T, :])
            eta_all = xin.tile([128, T, H], F16, tag="etaall")
            d_all = xin.tile([128, T, B], F16, tag="dall")
            for t in range(T):
                xt = x_all[:, t, :]
                st6 = stat.tile([128, 6], F32, tag="st6")
                nc.vector.bn_stats(st6[:], xt[:])
                mv = stat.tile([128, 2], F32, tag="mv")
                nc.vector.bn_aggr(mv[:], st6[:])
                sd = stat.tile([128, 1], F32, tag="sd")
                nc.scalar.activation(sd[:], mv[:, 1:2], mybir.ActivationFunctionType.Sqrt, bias=eps_t[:])
                rstd = stat.tile([128, 1], F32, tag="rstd")
                nc.vector.reciprocal(rstd[:], sd[:])
                eta = eta_all[:, t, :]
                nc.gpsimd.tensor_scalar(
                    eta, xt[:], mv[:, 0:1], rstd[:],
                    mybir.AluOpType.subtract, mybir.AluOpType.mult,
                )
                psT = psT_p.tile([128, H], F16, tag="psT", padded_shape=[128, 1024])
                nc.tensor.transpose(psT[:], eta, ident_t[:])
                etaT = work.tile([128, H], F16, tag="etaT")
                nc.scalar.activation(etaT[:], psT[:], mybir.ActivationFunctionType.Copy)
                psD = psD_p.tile([128, B], F32, tag="psD", padded_shape=[128, 512])
                nc.tensor.matmul(psD[:], etaT[:], dwT_t[:], start=True,
                                 stop=not cfg.use_bM)
                if cfg.use_bM:
                    nc.tensor.matmul(psD[:], ones1[:], c1_t[:], start=False, stop=True)
                nc.scalar.activation(d_all[:, t, :], psD[:], mybir.ActivationFunctionType.Copy)
                if cfg.use_pre_gb:
                    nc.vector.tensor_tensor(eta, eta, gb_t[:], mybir.AluOpType.mult)
                    nc.vector.tensor_tensor(eta, eta, bb_t[:], mybir.AluOpType.add)
            eta_dst = eta_out.ap().rearrange("p (t h) -> p t h", t=T)
            nc.sync.dma_start(eta_dst[:, 0:HT, :], eta_all[:, 0:HT, :])
            nc.sync.dma_start(eta_dst[:, HT:T, :], eta_all[:, HT:T, :])
            nc.scalar.dma_start(d_out.ap().rearrange("p (t b) -> p t b", t=T), d_all[:])

    nc.compile()
    return nc


def build_k2(cfg: Cfg):
    nc = bacc.Bacc("TRN2", target_bir_lowering=False, debug=False, num_devices=cfg.NCORES)
    H, B, T, G = cfg.H, cfg.B, cfg.T, cfg.G
    ngs = cfg.ngs
    NGmax = max(ngs)
    tot_ng = sum(ngs)
    # idx cols per tile: num_idxs_t/16 = ngs[t]*G*128/16 = ngs[t]*G*8
    tot_cols = tot_ng * G * 8

    table_in = nc.dram_tensor("table", [cfg.N + 1, 64], I32, kind="ExternalInput")
    eta_in = nc.dram_tensor("eta", [128, T * H], F16, kind="ExternalInput")
    idx_in = nc.dram_tensor("idx_plane", [128, tot_cols], I16, kind="ExternalInput")
    R_in = nc.dram_tensor("R_plane", [128, tot_ng], F32, kind="ExternalInput")
    iota_in = nc.dram_tensor("iota", [128, 128], F16, kind="ExternalInput")
    ident_in = nc.dram_tensor("ident", [128, 128], F16, kind="ExternalInput")
    upwT_in = nc.dram_tensor("upwT4", [128, H], F16, kind="ExternalInput")
    c_in = nc.dram_tensor("c_t", [1, H], F16, kind="ExternalInput")
    post_g_in = nc.dram_tensor("post_g", [1, H], F16, kind="ExternalInput")
    post_b_in = nc.dram_tensor("post_b", [1, H], F16, kind="ExternalInput")
    y_out = nc.dram_tensor("y", [128, T * H], F16, kind="ExternalOutput")

    with tile.TileContext(nc) as tc, ExitStack() as ctx:
        const = ctx.enter_context(tc.tile_pool(name="const", bufs=1))
        stat = ctx.enter_context(tc.tile_pool(name="stat", bufs=6))
        ohp = ctx.enter_context(tc.tile_pool(name="oh", bufs=14))
        gathp = ctx.enter_context(tc.tile_pool(name="gath", bufs=3))
        outp = ctx.enter_context(tc.tile_pool(name="outp", bufs=4))
        psAB = ctx.enter_context(tc.tile_pool(name="psAB", bufs=3, space="PSUM"))
        psTp = ctx.enter_context(tc.tile_pool(name="psT", bufs=2, space="PSUM"))
        psZp = ctx.enter_context(tc.tile_pool(name="psZ", bufs=2, space="PSUM"))

        iota_t = const.tile([128, 128], F16)
        nc.sync.dma_start(iota_t[:], iota_in[:])
        ident_t = const.tile([128, 128], F16)
        nc.sync.dma_start(ident_t[:], ident_in[:])
        Rp_t = const.tile([128, tot_ng], F32)
        nc.sync.dma_start(Rp_t[:], R_in[:])
        upwT_t = const.tile([128, H], F16)
        nc.sync.dma_start(upwT_t[:], upwT_in[:])
        eps_t = const.tile([128, 1], F32)
        nc.vector.memset(eps_t[:], EPS)
        idxp_t = const.tile([128, tot_cols], I16)
        HC = tot_cols // 2
        nc.sync.dma_start(idxp_t[:, 0:HC], idx_in[:, 0:HC])
        nc.scalar.dma_start(idxp_t[:, HC:tot_cols], idx_in[:, HC:tot_cols])
        if cfg.use_c:
            ones1 = const.tile([1, 128], F16)
            nc.vector.memset(ones1[:], 1.0)
            c_t = const.tile([1, H], F16)
            nc.sync.dma_start(c_t[:], c_in[:])
        if cfg.use_post_gb:
            ones_c = const.tile([1, 128], F16)
            nc.vector.memset(ones_c[:], 1.0)
            post_g_t = const.tile([1, H], F16)
            nc.sync.dma_start(post_g_t[:], post_g_in[:])
            post_b_t = const.tile([1, H], F16)
            nc.sync.dma_start(post_b_t[:], post_b_in[:])
            ps_g = psZp.tile([128, H], F32, tag="psZ", padded_shape=[128, 512])
            nc.tensor.matmul(ps_g[:], ones_c[:], post_g_t[:], start=True, stop=True)
            postg_b = const.tile([128, H], F16)
            nc.scalar.activation(postg_b[:], ps_g[:], mybir.ActivationFunctionType.Copy)
            ps_b = psZp.tile([128, H], F32, tag="psZ", padded_shape=[128, 512])
            nc.tensor.matmul(ps_b[:], ones_c[:], post_b_t[:], start=True, stop=True)
            postb_b = const.tile([128, H], F16)
            nc.scalar.activation(postb_b[:], ps_b[:], mybir.ActivationFunctionType.Copy)

        for _rep in range(cfg.reps):
            eta_all = const.tile([128, T, H], F16)
            nc.scalar.dma_start(eta_all[:], eta_in.ap().rearrange("p (t h) -> p t h", t=T))
            y_all = const.tile([128, T, H], F16)

            col0 = 0   # R_plane column base
            icol0 = 0  # idx_plane column base
            for t in range(T):
                NG = ngs[t]
                n_idx = NG * G * 128
                gath = gathp.tile([128, NGmax * G, 32], I32, tag="gath")
                _raw_dma_gather(
                    nc.gpsimd,
                    gath[:, 0 : NG * G, :],
                    table_in.ap()[:, 0:32],
                    idxp_t[:, icol0 : icol0 + n_idx // 16],
                    num_idxs=n_idx,
                    num_idxs_reg=n_idx,
                    elem_size=32,
                    elem_step=64,
                )
                gf = gath[:].bitcast(F16)  # [128, NGmax*G, 64]; cols 0:32 = d
                psA = psAB.tile([128, 128], F32, tag="psA", padded_shape=[128, 512])
                for g in range(NG):
                    oh = ohp.tile([128, 128], F16, tag="oh")
                    nc.vector.tensor_scalar(
                        oh[:], iota_t[:], Rp_t[:, col0 + g : col0 + g + 1], None,
                        mybir.AluOpType.is_equal,
                    )
                    s = g * G
                    # psA[m, je*32+c] += sum_k oh[k, m] * d[dst(g,k,je)][c]
                    nc.tensor.matmul(psA[:], oh[:], gf[:, s : s + G, 0:32],
                                     start=(g == 0), stop=(g == NG - 1))
                sbA = outp.tile([128, 128], F16, tag="sbA")
                nc.scalar.activation(sbA[:], psA[:], mybir.ActivationFunctionType.Copy)
                psT = psTp.tile([128, 128], F16, tag="psT", padded_shape=[128, 1024])
                nc.tensor.transpose(psT[:], sbA[:], ident_t[:])
                sbT = outp.tile([128, 128], F16, tag="sbT")
                nc.scalar.activation(sbT[:], psT[:], mybir.ActivationFunctionType.Copy)
                psZ = psZp.tile([128, H], F32, tag="psZ", padded_shape=[128, 512])
                # z2[m, h] = sum_{je,c} aggT[32*je+c, m] * upwT4[32*je+c, h]
                nc.tensor.matmul(psZ[:], sbT[:], upwT_t[:], start=True,
                                 stop=not cfg.use_c)
                if cfg.use_c:
                    nc.tensor.matmul(psZ[:], ones1[:], c_t[:], start=False, stop=True)
                # epilogue
                v = outp.tile([128, H], F16, tag="v")
                nc.scalar.activation(v[:], psZ[:], mybir.ActivationFunctionType.Relu)
                v2 = outp.tile([128, H], F16, tag="v2")
                nc.gpsimd.tensor_tensor(v2[:], v[:], eta_all[:, t, :], mybir.AluOpType.add)
                st6 = stat.tile([128, 6], F32, tag="st6")
                nc.vector.bn_stats(st6[:], v2[:])
                mv = stat.tile([128, 2], F32, tag="mv")
                nc.vector.bn_aggr(mv[:], st6[:])
                sd = stat.tile([128, 1], F32, tag="sd")
                nc.scalar.activation(sd[:], mv[:, 1:2], mybir.ActivationFunctionType.Sqrt, bias=eps_t[:])
                rstd = stat.tile([128, 1], F32, tag="rstd")
                nc.vector.reciprocal(rstd[:], sd[:])
                yt = y_all[:, t, :]
                nc.vector.tensor_scalar(
                    yt, v2[:], mv[:, 0:1], rstd[:],
                    mybir.AluOpType.subtract, mybir.AluOpType.mult,
                )
                if cfg.use_post_gb:
                    nc.vector.tensor_tensor(yt, yt, postg_b[:], mybir.AluOpType.mult)
                    nc.vector.tensor_tensor(yt, yt, postb_b[:], mybir.AluOpType.add)
                col0 += NG
                icol0 += n_idx // 16
            y_dst = y_out.ap().rearrange("p (t h) -> p t h", t=T)
            HT = T // 2
            nc.sync.dma_start(y_dst[:, 0:HT, :], y_all[:, 0:HT, :])
            nc.sync.dma_start(y_dst[:, HT:T, :], y_all[:, HT:T, :])

    nc.compile()
    return nc


# ---------------------------------------------------------------------------
# host-side prep
# ---------------------------------------------------------------------------


def prep_inputs(x, edge_index, down_w, down_b, up_w, up_b, pre_g, pre_b, post_g,
                post_b, cfg=None):
    N = x.shape[1]
    H = x.shape[2]
    B = down_w.shape[0]
    src = np.asarray(edge_index[0], dtype=np.int64)
    dst = np.asarray(edge_index[1], dtype=np.int64)

    if cfg is None:
        cfg = Cfg(N=N, H=H, B=B)
    G = cfg.G
    T = cfg.T
    n_tiles_total = N // 128

    order = np.argsort(src, kind="stable")
    src_s = src[order]
    dst_s = dst[order]
    tile_of = (src_s >> 7).astype(np.int64)
    row = (src_s & 127).astype(np.int64)
    cnt = np.bincount(src_s, minlength=N)
    row_start = np.concatenate([[0], np.cumsum(cnt)])
    pos_in_row = np.arange(len(src_s)) - row_start[src_s]
    claim_in_row = pos_in_row // G
    je = pos_in_row % G
    cpr = (cnt + G - 1) // G  # claims per src row
    cpr_t = cpr.reshape(n_tiles_total, 128)
    claim_base = np.cumsum(cpr_t, axis=1) - cpr_t  # within-tile claim offset per row
    claims_per_tile = cpr_t.sum(axis=1)
    claim_idx = claim_base[tile_of, row] + claim_in_row
    g_of = claim_idx // 128
    k_of = claim_idx % 128
    slot_of = (g_of * G + je) * 128 + k_of

    ng_per_tile = np.maximum(1, -(-claims_per_tile // 128)).reshape(cfg.NCORES, T)
    ngs = tuple(int(v) for v in ng_per_tile.max(axis=0))
    cfg.ngs = ngs
    cfg.use_bM = bool(np.any(pre_b != 0))
    cfg.use_c = bool(np.any(down_b != 0) or np.any(up_b != 0))
    cfg.use_pre_gb = bool(np.any(pre_g != 1) or np.any(pre_b != 0))
    cfg.use_post_gb = bool(np.any(post_g != 1) or np.any(post_b != 0))

    bounds = np.searchsorted(tile_of, np.arange(n_tiles_total + 1))
    tot_ng = sum(ngs)

    ident = np.eye(128, dtype=np.float16)
    iota = np.tile(np.arange(128, dtype=np.float16), (128, 1))
    dw_eff = (np.asarray(down_w, np.float32) * np.asarray(pre_g, np.float32)[None, :])
    dwT = np.ascontiguousarray(dw_eff.T).astype(np.float16)  # [H, B]
    c1 = (np.asarray(pre_b, np.float32) @ dw_eff.T).reshape(1, B).astype(np.float16)
    upwT4 = np.tile(np.asarray(up_w, np.float32).T, (128 // B, 1)).astype(np.float16)  # [128, H]
    c_t = (np.asarray(down_b, np.float32) @ np.asarray(up_w, np.float32).T
           + np.asarray(up_b, np.float32)).reshape(1, H).astype(np.float16)

    k1_maps, k2_maps = [], []
    for c in range(cfg.NCORES):
        fl_parts = []
        Rp = np.zeros((128, tot_ng), np.float32)
        col0 = 0
        for t in range(T):
            gt = c * T + t
            a, b = bounds[gt], bounds[gt + 1]
            n_slots = ngs[t] * G * 128
            fl = np.full(n_slots, N, dtype=np.int64)
            fl[slot_of[a:b]] = dst_s[a:b]
            fl_parts.append(np.tile(fl.reshape(-1, 16).T, (8, 1)))
            ncl = claims_per_tile[gt]
            rr = np.repeat(np.arange(128), cpr_t[gt])  # src row per claim
            ci = np.arange(ncl)
            Rp[ci % 128, col0 + ci // 128] = rr
            col0 += ngs[t]
        idx_plane = np.ascontiguousarray(
            np.concatenate(fl_parts, axis=1).astype(np.int16))

        k1_maps.append({
            "x_slab": np.ascontiguousarray(x[0, c * cfg.SLAB : (c + 1) * cfg.SLAB, :], dtype=np.float32),
            "ident": ident,
            "dwT": dwT,
            "c1": c1,
            "pre_g": np.asarray(pre_g, np.float16).reshape(1, H),
            "pre_b": np.asarray(pre_b, np.float16).reshape(1, H),
        })
        k2_maps.append({
            "idx_plane": idx_plane,
            "R_plane": np.ascontiguousarray(Rp),
            "iota": iota,
            "ident": ident,
            "upwT4": upwT4,
            "c_t": c_t,
            "post_g": np.asarray(post_g, np.float16).reshape(1, H),
            "post_b": np.asarray(post_b, np.float16).reshape(1, H),
        })
    return cfg, k1_maps, k2_maps


def table_from_d(cfg, d_list):
    """d_list: per-core [128, T*B] f16 -> int32 table [N+1, 64] (256B rows)."""
    B = cfg.B
    parts = []
    for arr in d_list:
        a = np.asarray(arr).astype(np.float16).reshape(128, cfg.T, B)
        parts.append(np.ascontiguousarray(a.transpose(1, 0, 2)).reshape(cfg.SLAB, B))
    d_full = np.concatenate(parts, axis=0)
    tabf = np.zeros((cfg.N + 1, 128), np.float16)
    tabf[: cfg.N, :B] = d_full
    return tabf.view(np.int32)


def y_from_outs(cfg, y_list):
    """per-core [128, T*H] f16 -> [1, N, H] f32."""
    H = cfg.H
    parts = []
    for arr in y_list:
        a = np.asarray(arr).astype(np.float32).reshape(128, cfg.T, H)
        parts.append(np.ascontiguousarray(a.transpose(1, 0, 2)).reshape(cfg.SLAB, H))
    return np.concatenate(parts, axis=0)[None]


# ---------------------------------------------------------------------------
# main entry
# ---------------------------------------------------------------------------

_CACHE = {}


def _run_spmd(nc, maps, cores):
    # int64 table input requires x64 through the bass2jax/PJRT path
    try:
        import jax
        jax.config.update("jax_enable_x64", True)
    except Exception:
        pass
    from concourse.bass_utils import run_bass_kernel_spmd

    last_err = None
    for _attempt in range(3):
        try:
            return run_bass_kernel_spmd(nc, maps, cores).results
        except Exception as e:  # transient device/transport errors
            last_err = e
            import time as _time
            _time.sleep(2.0)
    raise last_err


def kernel(x, edge_index, down_w, down_b, up_w, up_b, pre_g, pre_b, post_g, post_b):
    import numpy as _np

    inputs = dict(x=_np.asarray(x), edge_index=_np.asarray(edge_index),
                  down_w=_np.asarray(down_w), down_b=_np.asarray(down_b),
                  up_w=_np.asarray(up_w), up_b=_np.asarray(up_b),
                  pre_g=_np.asarray(pre_g), pre_b=_np.asarray(pre_b),
                  post_g=_np.asarray(post_g), post_b=_np.asarray(post_b))
    cfg, k1_maps, k2_maps = prep_inputs(**inputs)
    key = (cfg.N, cfg.H, cfg.B, cfg.G, cfg.ngs, cfg.use_bM, cfg.use_c,
           cfg.use_pre_gb, cfg.use_post_gb)
    if key not in _CACHE:
        _CACHE[key] = (build_k1(cfg), build_k2(cfg))
    nc1, nc2 = _CACHE[key]
    cores = list(range(cfg.NCORES))
    r1 = _run_spmd(nc1, k1_maps, cores)
    table = table_from_d(cfg, [r1[c]["d"] for c in range(cfg.NCORES)])
    for c in range(cfg.NCORES):
        k2_maps[c]["table"] = table
        k2_maps[c]["eta"] = r1[c]["eta"]
    r2 = _run_spmd(nc2, k2_maps, cores)
    return y_from_outs(cfg, [r2[c]["y"] for c in range(cfg.NCORES)]).astype(_np.float32)


# revision 38
# speedup vs baseline: 1.8560x; 1.0291x over previous
"""GAdapter (GNN message passing + adapter MLP) Bass kernel for Trainium2, 8-core SPMD.

Entry point: kernel(**inputs) -> np.ndarray [1, N, H] float32.

Two-launch design (no collectives), aggregation in the down-projected space.

k1 (per core): LN of own x slab -> eta (residual, f16) and
    d = eta @ (diag(pre_g) down_w^T) [+ pre_b down_w^T]   [SLAB, 32] f32
Host: concat d slabs -> table [N+1, 64] f32 (cols 32:64 zero, last row zero
    sentinel), viewed as int64 [N+1, 32]; replicate to all cores.
k2 (per core): per 128-row tile, gather table rows (256B each = one idx per
    edge; int64 typing keeps the gather's free-size small) for group-packed
    edge slots. Per claim group g one f16 one-hot (stationary lhsT) times the
    f16 payload view (moving, strided slice [slots, 4, 0:32] skips the pad)
    accumulates psum[m, je*32+c]. Then one PE transpose and ONE K=128
    up-projection matmul against up_w^T stacked 4x gives z2 = agg @ up_w^T
    (the sum over edge-positions je happens inside the contraction);
    relu + residual + post-LN; DMA out.

Claim packing: a claim is <= G=4 edges sharing one source row within the
tile; claim i of a tile sits at slot (g=i//128, k=i%128); its 4 edges'
gathers land at out[k, g*4+je].
"""

from contextlib import ExitStack
from dataclasses import dataclass, field

import numpy as np

import concourse.bass as bass
import concourse.tile as tile
from concourse import bacc, mybir

F32 = mybir.dt.float32
F32R = mybir.dt.float32r
F16 = mybir.dt.float16
I64 = mybir.dt.int64
I32 = mybir.dt.int32
I16 = mybir.dt.int16
EPS = 1e-5


def _raw_dma_gather(g, out_ap, in_ap, idxs_ap, num_idxs, num_idxs_reg, elem_size,
                    elem_step, single_packet=False, queue_num=0):
    """dma_gather for sub-256B elements (elem read < 256B row stride).

    Same lowering as bass's dma_gather non-transpose DRAM path, minus the
    elem_size%256 assert (which only the transpose mode needs). The row
    stride (elem_step * dtype) must still be a 256B multiple.
    """
    from concourse.bass import exact_div

    stride_bytes = elem_step * mybir.dt.size(in_ap.dtype)
    stride_bytes_256 = exact_div(stride_bytes, 256)
    _in_ap = g.lower_ap_dma(in_ap, for_custom_bir_dma=True)
    _idxs_ap = g.lower_ap(idxs_ap)
    _out_ap = g.lower_ap(out_ap)
    return g.add_instruction(
        mybir.InstDMAGatherAnt(
            name=g.bass.get_next_instruction_name(),
            ins=[*_in_ap, _idxs_ap, g.lower_val_access(g.to_reg(num_idxs_reg))],
            outs=[_out_ap], transpose=False, num_idxs=num_idxs,
            elem_size=elem_size, stride_bytes_256=stride_bytes_256, gen_mode=0,
            single_packet=single_packet, queue_num=queue_num,
            sbuf_tokens_per_rank=0, sbuf_free_dim_per_rank=0,
            sbuf_free_dim_pad_per_rank=0, sbuf_byte_offset=0,
        ))


@dataclass
class Cfg:
    N: int = 16384
    H: int = 128
    B: int = 32
    NCORES: int = 8
    G: int = 4
    ngs: tuple = field(default_factory=lambda: tuple([9] * 16))  # per-tile claim groups
    use_bM: bool = False       # pre_b != 0 (bias into d)
    use_pre_gb: bool = False   # pre_g/pre_b non-identity (residual adjust)
    use_c: bool = False        # down_b/up_b != 0
    use_post_gb: bool = False  # post_g/post_b non-identity
    reps: int = 1

    @property
    def SLAB(self):
        return self.N // self.NCORES

    @property
    def T(self):
        return self.SLAB // 128


def build_k1(cfg: Cfg):
    nc = bacc.Bacc("TRN2", target_bir_lowering=False, debug=False, num_devices=cfg.NCORES)
    H, B, T = cfg.H, cfg.B, cfg.T
    x_slab = nc.dram_tensor("x_slab", [cfg.SLAB, H], F32, kind="ExternalInput")
    ident_in = nc.dram_tensor("ident", [128, 128], F16, kind="ExternalInput")
    dwT_in = nc.dram_tensor("dwT", [H, B], F16, kind="ExternalInput")
    c1_in = nc.dram_tensor("c1", [1, B], F16, kind="ExternalInput")
    pre_g_in = nc.dram_tensor("pre_g", [1, H], F16, kind="ExternalInput")
    pre_b_in = nc.dram_tensor("pre_b", [1, H], F16, kind="ExternalInput")
    d_out = nc.dram_tensor("d", [128, T * B], F16, kind="ExternalOutput")
    eta_out = nc.dram_tensor("eta", [128, T * H], F16, kind="ExternalOutput")

    with tile.TileContext(nc) as tc, ExitStack() as ctx:
        const = ctx.enter_context(tc.tile_pool(name="const", bufs=1))
        xin = ctx.enter_context(tc.tile_pool(name="xin", bufs=1))
        stat = ctx.enter_context(tc.tile_pool(name="stat", bufs=6))
        work = ctx.enter_context(tc.tile_pool(name="work", bufs=3))
        psT_p = ctx.enter_context(tc.tile_pool(name="psT", bufs=3, space="PSUM"))
        psD_p = ctx.enter_context(tc.tile_pool(name="psD", bufs=3, space="PSUM"))

        ident_t = const.tile([128, 128], F16)
        nc.sync.dma_start(ident_t[:], ident_in[:])
        dwT_t = const.tile([H, B], F16)
        nc.sync.dma_start(dwT_t[:], dwT_in[:])
        eps_t = const.tile([128, 1], F32)
        nc.vector.memset(eps_t[:], EPS)
        if cfg.use_bM:
            ones1 = const.tile([1, 128], F16)
            nc.vector.memset(ones1[:], 1.0)
            c1_t = const.tile([1, B], F16)
            nc.sync.dma_start(c1_t[:], c1_in[:])
        if cfg.use_pre_gb:
            pre_g_t = const.tile([1, H], F16)
            nc.sync.dma_start(pre_g_t[:], pre_g_in[:])
            pre_b_t = const.tile([1, H], F16)
            nc.sync.dma_start(pre_b_t[:], pre_b_in[:])
            ones_c = const.tile([1, 128], F16)
            nc.vector.memset(ones_c[:], 1.0)
            ps_g = psT_p.tile([128, H], F32, tag="pro", padded_shape=[128, 512])
            nc.tensor.matmul(ps_g[:], ones_c[:], pre_g_t[:], start=True, stop=True)
            gb_t = const.tile([128, H], F16)
            nc.scalar.activation(gb_t[:], ps_g[:], mybir.ActivationFunctionType.Copy)
            ps_b = psT_p.tile([128, H], F32, tag="pro", padded_shape=[128, 512])
            nc.tensor.matmul(ps_b[:], ones_c[:], pre_b_t[:], start=True, stop=True)
            bb_t = const.tile([128, H], F16)
            nc.scalar.activation(bb_t[:], ps_b[:], mybir.ActivationFunctionType.Copy)

        for _rep in range(cfg.reps):
            x_all = xin.tile([128, T, H], F32, tag="xall")
            x_src = x_slab.ap().rearrange("(t p) h -> p t h", p=128)
            HT = T // 2
            nc.sync.dma_start(x_all[:, 0:HT, :], x_src[:, 0:HT, :])
            nc.gpsimd.dma_start(x_all[:, HT:T, :], x_src[:, HT:T, :])
            eta_all = xin.tile([128, T, H], F16, tag="etaall")
            d_all = xin.tile([128, T, B], F16, tag="dall")
            live = {}

            def stage_a(t):
                xt = x_all[:, t, :]
                st6 = stat.tile([128, 6], F32, tag="st6", name="st6")
                nc.vector.bn_stats(st6[:], xt[:])
                mv = stat.tile([128, 2], F32, tag="mv", name="mv")
                nc.vector.bn_aggr(mv[:], st6[:])
                sd = stat.tile([128, 1], F32, tag="sd", name="sd")
                nc.scalar.activation(sd[:], mv[:, 1:2],
                                     mybir.ActivationFunctionType.Sqrt, bias=eps_t[:])
                rstd = stat.tile([128, 1], F32, tag="rstd", name="rstd")
                nc.vector.reciprocal(rstd[:], sd[:])
                live[t] = {"mv": mv, "rstd": rstd}

            def stage_b(t):
                mv, rstd = live[t]["mv"], live[t]["rstd"]
                eta = eta_all[:, t, :]
                nc.gpsimd.tensor_scalar(
                    eta, x_all[:, t, :], mv[:, 0:1], rstd[:],
                    mybir.AluOpType.subtract, mybir.AluOpType.mult,
                )
                psT = psT_p.tile([128, H], F16, tag="psT", padded_shape=[128, 1024], name="psT")
                nc.tensor.transpose(psT[:], eta, ident_t[:])
                etaT = work.tile([128, H], F16, tag="etaT", name="etaT")
                nc.scalar.activation(etaT[:], psT[:], mybir.ActivationFunctionType.Copy)
                live[t]["etaT"] = etaT

            def stage_c(t):
                etaT = live[t]["etaT"]
                psD = psD_p.tile([128, B], F32, tag="psD", padded_shape=[128, 512], name="psD")
                nc.tensor.matmul(psD[:], etaT[:], dwT_t[:], start=True,
                                 stop=not cfg.use_bM)
                if cfg.use_bM:
                    nc.tensor.matmul(psD[:], ones1[:], c1_t[:], start=False, stop=True)
                nc.vector.tensor_copy(d_all[:, t, :], psD[:])
                if cfg.use_pre_gb:
                    eta = eta_all[:, t, :]
                    nc.vector.tensor_tensor(eta, eta, gb_t[:], mybir.AluOpType.mult)
                    nc.vector.tensor_tensor(eta, eta, bb_t[:], mybir.AluOpType.add)
                del live[t]

            for tt in range(T + 2):
                if tt < T:
                    stage_a(tt)
                if 1 <= tt <= T:
                    stage_b(tt - 1)
                if tt >= 2:
                    stage_c(tt - 2)
            eta_dst = eta_out.ap().rearrange("p (t h) -> p t h", t=T)
            nc.sync.dma_start(eta_dst[:, 0:HT, :], eta_all[:, 0:HT, :])
            nc.sync.dma_start(eta_dst[:, HT:T, :], eta_all[:, HT:T, :])
            nc.sync.dma_start(d_out.ap().rearrange("p (t b) -> p t b", t=T), d_all[:])

    nc.compile()
    return nc


def build_k2(cfg: Cfg):
    nc = bacc.Bacc("TRN2", target_bir_lowering=False, debug=False, num_devices=cfg.NCORES)
    H, B, T, G = cfg.H, cfg.B, cfg.T, cfg.G
    ngs = cfg.ngs
    NGmax = max(ngs)
    tot_ng = sum(ngs)
    # idx cols per tile: num_idxs_t/16 = ngs[t]*G*128/16 = ngs[t]*G*8
    tot_cols = tot_ng * G * 8

    table_in = nc.dram_tensor("table", [cfg.N + 1, 64], I32, kind="ExternalInput")
    eta_in = nc.dram_tensor("eta", [128, T * H], F16, kind="ExternalInput")
    idx_in = nc.dram_tensor("idx_plane", [128, tot_cols], I16, kind="ExternalInput")
    R_in = nc.dram_tensor("R_plane", [128, tot_ng], F32, kind="ExternalInput")
    iota_in = nc.dram_tensor("iota", [128, 128], F16, kind="ExternalInput")
    ident_in = nc.dram_tensor("ident", [128, 128], F16, kind="ExternalInput")
    upwT_in = nc.dram_tensor("upwT4", [128, H], F16, kind="ExternalInput")
    c_in = nc.dram_tensor("c_t", [1, H], F16, kind="ExternalInput")
    post_g_in = nc.dram_tensor("post_g", [1, H], F16, kind="ExternalInput")
    post_b_in = nc.dram_tensor("post_b", [1, H], F16, kind="ExternalInput")
    y_out = nc.dram_tensor("y", [128, T * H], F16, kind="ExternalOutput")

    with tile.TileContext(nc) as tc, ExitStack() as ctx:
        const = ctx.enter_context(tc.tile_pool(name="const", bufs=1))
        stat = ctx.enter_context(tc.tile_pool(name="stat", bufs=8))
        ohp = ctx.enter_context(tc.tile_pool(name="oh", bufs=24))
        gathp = ctx.enter_context(tc.tile_pool(name="gath", bufs=3))
        outp = ctx.enter_context(tc.tile_pool(name="outp", bufs=6))
        psAB = ctx.enter_context(tc.tile_pool(name="psAB", bufs=3, space="PSUM"))
        psTp = ctx.enter_context(tc.tile_pool(name="psT", bufs=2, space="PSUM"))
        psZp = ctx.enter_context(tc.tile_pool(name="psZ", bufs=3, space="PSUM"))

        iota_t = const.tile([128, 128], F16)
        nc.sync.dma_start(iota_t[:], iota_in[:])
        ident_t = const.tile([128, 128], F16)
        nc.sync.dma_start(ident_t[:], ident_in[:])
        Rp_t = const.tile([128, tot_ng], F32)
        nc.sync.dma_start(Rp_t[:], R_in[:])
        upwT_t = const.tile([128, H], F16)
        nc.sync.dma_start(upwT_t[:], upwT_in[:])
        eps_t = const.tile([128, 1], F32)
        nc.vector.memset(eps_t[:], EPS)
        idxp_t = const.tile([128, tot_cols], I16)
        HC = tot_cols // 2
        nc.sync.dma_start(idxp_t[:, 0:HC], idx_in[:, 0:HC])
        nc.sync.dma_start(idxp_t[:, HC:tot_cols], idx_in[:, HC:tot_cols])
        if cfg.use_c:
            ones1 = const.tile([1, 128], F16)
            nc.vector.memset(ones1[:], 1.0)
            c_t = const.tile([1, H], F16)
            nc.sync.dma_start(c_t[:], c_in[:])
        if cfg.use_post_gb:
            ones_c = const.tile([1, 128], F16)
            nc.vector.memset(ones_c[:], 1.0)
            post_g_t = const.tile([1, H], F16)
            nc.sync.dma_start(post_g_t[:], post_g_in[:])
            post_b_t = const.tile([1, H], F16)
            nc.sync.dma_start(post_b_t[:], post_b_in[:])
            ps_g = psZp.tile([128, H], F32, tag="psZ", padded_shape=[128, 512])
            nc.tensor.matmul(ps_g[:], ones_c[:], post_g_t[:], start=True, stop=True)
            postg_b = const.tile([128, H], F16)
            nc.scalar.activation(postg_b[:], ps_g[:], mybir.ActivationFunctionType.Copy)
            ps_b = psZp.tile([128, H], F32, tag="psZ", padded_shape=[128, 512])
            nc.tensor.matmul(ps_b[:], ones_c[:], post_b_t[:], start=True, stop=True)
            postb_b = const.tile([128, H], F16)
            nc.scalar.activation(postb_b[:], ps_b[:], mybir.ActivationFunctionType.Copy)

        for _rep in range(cfg.reps):
            eta_all = const.tile([128, T, H], F16)
            nc.sync.dma_start(eta_all[:], eta_in.ap().rearrange("p (t h) -> p t h", t=T))
            y_all = const.tile([128, T, H], F16)

            colb = [sum(ngs[:i]) for i in range(T)]
            icolb = [sum(ngs[:i]) * G * 8 for i in range(T)]
            live = {}  # t -> dict of tiles crossing stage boundaries

            def stage_a(t):
                NG = ngs[t]
                n_idx = NG * G * 128
                gath = gathp.tile([128, NGmax * G, 32], I32, tag="gath", name="gath")
                _raw_dma_gather(
                    nc.gpsimd,
                    gath[:, 0 : NG * G, :],
                    table_in.ap()[:, 0:32],
                    idxp_t[:, icolb[t] : icolb[t] + n_idx // 16],
                    num_idxs=n_idx,
                    num_idxs_reg=n_idx,
                    elem_size=32,
                    elem_step=64,
                )
                ohs = []
                for g in range(NG):
                    oh = ohp.tile([128, 128], F16, tag="oh", name="oh")
                    nc.vector.tensor_scalar(
                        oh[:], iota_t[:], Rp_t[:, colb[t] + g : colb[t] + g + 1],
                        None, mybir.AluOpType.is_equal,
                    )
                    ohs.append(oh)
                live[t] = {"gath": gath, "ohs": ohs}

            def stage_b(t):
                NG = ngs[t]
                gath = live[t]["gath"]
                ohs = live[t]["ohs"]
                gf = gath[:].bitcast(F16)  # [128, NGmax*G, 64]; cols 0:32 = d
                psA = psAB.tile([128, 128], F32, tag="psA", padded_shape=[128, 512], name="psA")
                for g in range(NG):
                    # psA[m, je*32+c] += sum_k oh[k, m] * d[dst(g,k,je)][c]
                    nc.tensor.matmul(psA[:], ohs[g][:], gf[:, g * G : g * G + G, 0:32],
                                     start=(g == 0), stop=(g == NG - 1))
                sbA = outp.tile([128, 128], F16, tag="sbA", name="sbA")
                nc.scalar.activation(sbA[:], psA[:], mybir.ActivationFunctionType.Copy)
                psT = psTp.tile([128, 128], F16, tag="psT", padded_shape=[128, 1024], name="psT")
                nc.tensor.transpose(psT[:], sbA[:], ident_t[:])
                sbT = outp.tile([128, 128], F16, tag="sbT", name="sbT")
                nc.vector.tensor_copy(sbT[:], psT[:])
                psZ = psZp.tile([128, H], F32, tag="psZ", padded_shape=[128, 512], name="psZ")
                # z2[m, h] = sum_{je,c} aggT[32*je+c, m] * upwT4[32*je+c, h]
                nc.tensor.matmul(psZ[:], sbT[:], upwT_t[:], start=True,
                                 stop=not cfg.use_c)
                if cfg.use_c:
                    nc.tensor.matmul(psZ[:], ones1[:], c_t[:], start=False, stop=True)
                live[t]["psZ"] = psZ

            def stage_c(t):
                psZ = live[t]["psZ"]
                v = outp.tile([128, H], F16, tag="v", name="v")
                nc.scalar.activation(v[:], psZ[:], mybir.ActivationFunctionType.Relu)
                v2 = outp.tile([128, H], F16, tag="v2", name="v2")
                nc.gpsimd.tensor_tensor(v2[:], v[:], eta_all[:, t, :], mybir.AluOpType.add)
                st6 = stat.tile([128, 6], F32, tag="st6", name="st6")
                nc.vector.bn_stats(st6[:], v2[:])
                mv = stat.tile([128, 2], F32, tag="mv", name="mv")
                nc.vector.bn_aggr(mv[:], st6[:])
                sd = stat.tile([128, 1], F32, tag="sd", name="sd")
                nc.scalar.activation(sd[:], mv[:, 1:2],
                                     mybir.ActivationFunctionType.Sqrt, bias=eps_t[:])
                rstd = stat.tile([128, 1], F32, tag="rstd", name="rstd")
                nc.vector.reciprocal(rstd[:], sd[:])
                yt = y_all[:, t, :]
                nc.vector.tensor_scalar(
                    yt, v2[:], mv[:, 0:1], rstd[:],
                    mybir.AluOpType.subtract, mybir.AluOpType.mult,
                )
                if cfg.use_post_gb:
                    nc.vector.tensor_tensor(yt, yt, postg_b[:], mybir.AluOpType.mult)
                    nc.vector.tensor_tensor(yt, yt, postb_b[:], mybir.AluOpType.add)
                del live[t]

            for tt in range(T + 2):
                if tt < T:
                    stage_a(tt)
                if 1 <= tt <= T:
                    stage_b(tt - 1)
                if tt >= 2:
                    stage_c(tt - 2)
            y_dst = y_out.ap().rearrange("p (t h) -> p t h", t=T)
            HT = T // 2
            nc.sync.dma_start(y_dst[:, 0:HT, :], y_all[:, 0:HT, :])
            nc.sync.dma_start(y_dst[:, HT:T, :], y_all[:, HT:T, :])

    nc.compile()
    return nc


# ---------------------------------------------------------------------------
# host-side prep
# ---------------------------------------------------------------------------


def prep_inputs(x, edge_index, down_w, down_b, up_w, up_b, pre_g, pre_b, post_g,
                post_b, cfg=None):
    N = x.shape[1]
    H = x.shape[2]
    B = down_w.shape[0]
    src = np.asarray(edge_index[0], dtype=np.int64)
    dst = np.asarray(edge_index[1], dtype=np.int64)

    if cfg is None:
        cfg = Cfg(N=N, H=H, B=B)
    G = cfg.G
    T = cfg.T
    n_tiles_total = N // 128

    order = np.argsort(src, kind="stable")
    src_s = src[order]
    dst_s = dst[order]
    tile_of = (src_s >> 7).astype(np.int64)
    row = (src_s & 127).astype(np.int64)
    cnt = np.bincount(src_s, minlength=N)
    row_start = np.concatenate([[0], np.cumsum(cnt)])
    pos_in_row = np.arange(len(src_s)) - row_start[src_s]
    claim_in_row = pos_in_row // G
    je = pos_in_row % G
    cpr = (cnt + G - 1) // G  # claims per src row
    cpr_t = cpr.reshape(n_tiles_total, 128)
    claim_base = np.cumsum(cpr_t, axis=1) - cpr_t  # within-tile claim offset per row
    claims_per_tile = cpr_t.sum(axis=1)
    claim_idx = claim_base[tile_of, row] + claim_in_row
    g_of = claim_idx // 128
    k_of = claim_idx % 128
    slot_of = (g_of * G + je) * 128 + k_of

    ng_per_tile = np.maximum(1, -(-claims_per_tile // 128)).reshape(cfg.NCORES, T)
    ngs = tuple(int(v) for v in ng_per_tile.max(axis=0))
    cfg.ngs = ngs
    cfg.use_bM = bool(np.any(pre_b != 0))
    cfg.use_c = bool(np.any(down_b != 0) or np.any(up_b != 0))
    cfg.use_pre_gb = bool(np.any(pre_g != 1) or np.any(pre_b != 0))
    cfg.use_post_gb = bool(np.any(post_g != 1) or np.any(post_b != 0))

    bounds = np.searchsorted(tile_of, np.arange(n_tiles_total + 1))
    tot_ng = sum(ngs)

    ident = np.eye(128, dtype=np.float16)
    iota = np.tile(np.arange(128, dtype=np.float16), (128, 1))
    dw_eff = (np.asarray(down_w, np.float32) * np.asarray(pre_g, np.float32)[None, :])
    dwT = np.ascontiguousarray(dw_eff.T).astype(np.float16)  # [H, B]
    c1 = (np.asarray(pre_b, np.float32) @ dw_eff.T).reshape(1, B).astype(np.float16)
    upwT4 = np.tile(np.asarray(up_w, np.float32).T, (128 // B, 1)).astype(np.float16)  # [128, H]
    c_t = (np.asarray(down_b, np.float32) @ np.asarray(up_w, np.float32).T
           + np.asarray(up_b, np.float32)).reshape(1, H).astype(np.float16)

    k1_maps, k2_maps = [], []
    for c in range(cfg.NCORES):
        fl_parts = []
        Rp = np.zeros((128, tot_ng), np.float32)
        col0 = 0
        for t in range(T):
            gt = c * T + t
            a, b = bounds[gt], bounds[gt + 1]
            n_slots = ngs[t] * G * 128
            fl = np.full(n_slots, N, dtype=np.int64)
            fl[slot_of[a:b]] = dst_s[a:b]
            fl_parts.append(np.tile(fl.reshape(-1, 16).T, (8, 1)))
            ncl = claims_per_tile[gt]
            rr = np.repeat(np.arange(128), cpr_t[gt])  # src row per claim
            ci = np.arange(ncl)
            Rp[ci % 128, col0 + ci // 128] = rr
            col0 += ngs[t]
        idx_plane = np.ascontiguousarray(
            np.concatenate(fl_parts, axis=1).astype(np.int16))

        k1_maps.append({
            "x_slab": np.ascontiguousarray(x[0, c * cfg.SLAB : (c + 1) * cfg.SLAB, :], dtype=np.float32),
            "ident": ident,
            "dwT": dwT,
            "c1": c1,
            "pre_g": np.asarray(pre_g, np.float16).reshape(1, H),
            "pre_b": np.asarray(pre_b, np.float16).reshape(1, H),
        })
        k2_maps.append({
            "idx_plane": idx_plane,
            "R_plane": np.ascontiguousarray(Rp),
            "iota": iota,
            "ident": ident,
            "upwT4": upwT4,
            "c_t": c_t,
            "post_g": np.asarray(post_g, np.float16).reshape(1, H),
            "post_b": np.asarray(post_b, np.float16).reshape(1, H),
        })
    return cfg, k1_maps, k2_maps


def table_from_d(cfg, d_list):
    """d_list: per-core [128, T*B] f16 -> int32 table [N+1, 64] (256B rows)."""
    B = cfg.B
    parts = []
    for arr in d_list:
        a = np.asarray(arr).astype(np.float16).reshape(128, cfg.T, B)
        parts.append(np.ascontiguousarray(a.transpose(1, 0, 2)).reshape(cfg.SLAB, B))
    d_full = np.concatenate(parts, axis=0)
    tabf = np.zeros((cfg.N + 1, 128), np.float16)
    tabf[: cfg.N, :B] = d_full
    return tabf.view(np.int32)


def y_from_outs(cfg, y_list):
    """per-core [128, T*H] f16 -> [1, N, H] f32."""
    H = cfg.H
    parts = []
    for arr in y_list:
        a = np.asarray(arr).astype(np.float32).reshape(128, cfg.T, H)
        parts.append(np.ascontiguousarray(a.transpose(1, 0, 2)).reshape(cfg.SLAB, H))
    return np.concatenate(parts, axis=0)[None]


# ---------------------------------------------------------------------------
# main entry
# ---------------------------------------------------------------------------

_CACHE = {}


def _run_spmd(nc, maps, cores):
    # int64 table input requires x64 through the bass2jax/PJRT path
    try:
        import jax
        jax.config.update("jax_enable_x64", True)
    except Exception:
        pass
    from concourse.bass_utils import run_bass_kernel_spmd

    last_err = None
    for _attempt in range(3):
        try:
            return run_bass_kernel_spmd(nc, maps, cores).results
        except Exception as e:  # transient device/transport errors
            last_err = e
            import time as _time
            _time.sleep(2.0)
    raise last_err


def kernel(x, edge_index, down_w, down_b, up_w, up_b, pre_g, pre_b, post_g, post_b):
    import numpy as _np

    inputs = dict(x=_np.asarray(x), edge_index=_np.asarray(edge_index),
                  down_w=_np.asarray(down_w), down_b=_np.asarray(down_b),
                  up_w=_np.asarray(up_w), up_b=_np.asarray(up_b),
                  pre_g=_np.asarray(pre_g), pre_b=_np.asarray(pre_b),
                  post_g=_np.asarray(post_g), post_b=_np.asarray(post_b))
    cfg, k1_maps, k2_maps = prep_inputs(**inputs)
    key = (cfg.N, cfg.H, cfg.B, cfg.G, cfg.ngs, cfg.use_bM, cfg.use_c,
           cfg.use_pre_gb, cfg.use_post_gb)
    if key not in _CACHE:
        _CACHE[key] = (build_k1(cfg), build_k2(cfg))
    nc1, nc2 = _CACHE[key]
    cores = list(range(cfg.NCORES))
    r1 = _run_spmd(nc1, k1_maps, cores)
    table = table_from_d(cfg, [r1[c]["d"] for c in range(cfg.NCORES)])
    for c in range(cfg.NCORES):
        k2_maps[c]["table"] = table
        k2_maps[c]["eta"] = r1[c]["eta"]
    r2 = _run_spmd(nc2, k2_maps, cores)
    return y_from_outs(cfg, [r2[c]["y"] for c in range(cfg.NCORES)]).astype(_np.float32)


# revision 39
# speedup vs baseline: 2.0921x; 1.1272x over previous
"""GAdapter (GNN message passing + adapter MLP) Bass kernel for Trainium2, 8-core SPMD.

Entry point: kernel(**inputs) -> np.ndarray [1, N, H] float32.

Two-launch design (no collectives), aggregation in the down-projected space.

k1 (per core): LN of own x slab -> eta (residual, f16) and
    d = eta @ (diag(pre_g) down_w^T) [+ pre_b down_w^T]   [SLAB, 32] f32
Host: concat d slabs -> table [N+1, 64] f32 (cols 32:64 zero, last row zero
    sentinel), viewed as int64 [N+1, 32]; replicate to all cores.
k2 (per core): per 128-row tile, gather table rows (256B each = one idx per
    edge; int64 typing keeps the gather's free-size small) for group-packed
    edge slots. Per claim group g one f16 one-hot (stationary lhsT) times the
    f16 payload view (moving, strided slice [slots, 4, 0:32] skips the pad)
    accumulates psum[m, je*32+c]. Then one PE transpose and ONE K=128
    up-projection matmul against up_w^T stacked 4x gives z2 = agg @ up_w^T
    (the sum over edge-positions je happens inside the contraction);
    relu + residual + post-LN; DMA out.

Claim packing: a claim is <= G=4 edges sharing one source row within the
tile; claim i of a tile sits at slot (g=i//128, k=i%128); its 4 edges'
gathers land at out[k, g*4+je].
"""

from contextlib import ExitStack
from dataclasses import dataclass, field

import numpy as np

import concourse.bass as bass
import concourse.tile as tile
from concourse import bacc, mybir

F32 = mybir.dt.float32
F32R = mybir.dt.float32r
F16 = mybir.dt.float16
I64 = mybir.dt.int64
I32 = mybir.dt.int32
I16 = mybir.dt.int16
EPS = 1e-5


def _raw_dma_gather(g, out_ap, in_ap, idxs_ap, num_idxs, num_idxs_reg, elem_size,
                    elem_step, single_packet=False, queue_num=0):
    """dma_gather for sub-256B elements (elem read < 256B row stride).

    Same lowering as bass's dma_gather non-transpose DRAM path, minus the
    elem_size%256 assert (which only the transpose mode needs). The row
    stride (elem_step * dtype) must still be a 256B multiple.
    """
    from concourse.bass import exact_div

    stride_bytes = elem_step * mybir.dt.size(in_ap.dtype)
    stride_bytes_256 = exact_div(stride_bytes, 256)
    _in_ap = g.lower_ap_dma(in_ap, for_custom_bir_dma=True)
    _idxs_ap = g.lower_ap(idxs_ap)
    _out_ap = g.lower_ap(out_ap)
    return g.add_instruction(
        mybir.InstDMAGatherAnt(
            name=g.bass.get_next_instruction_name(),
            ins=[*_in_ap, _idxs_ap, g.lower_val_access(g.to_reg(num_idxs_reg))],
            outs=[_out_ap], transpose=False, num_idxs=num_idxs,
            elem_size=elem_size, stride_bytes_256=stride_bytes_256, gen_mode=0,
            single_packet=single_packet, queue_num=queue_num,
            sbuf_tokens_per_rank=0, sbuf_free_dim_per_rank=0,
            sbuf_free_dim_pad_per_rank=0, sbuf_byte_offset=0,
        ))


@dataclass
class Cfg:
    N: int = 16384
    H: int = 128
    B: int = 32
    NCORES: int = 8
    G: int = 4
    ngs: tuple = field(default_factory=lambda: tuple([9] * 16))  # per-tile claim groups
    use_bM: bool = False       # pre_b != 0 (bias into d)
    use_pre_gb: bool = False   # pre_g/pre_b non-identity (residual adjust)
    use_c: bool = False        # down_b/up_b != 0
    use_post_gb: bool = False  # post_g/post_b non-identity
    reps: int = 1

    @property
    def SLAB(self):
        return self.N // self.NCORES

    @property
    def T(self):
        return self.SLAB // 128


def build_k1(cfg: Cfg):
    nc = bacc.Bacc("TRN2", target_bir_lowering=False, debug=False, num_devices=cfg.NCORES)
    H, B, T = cfg.H, cfg.B, cfg.T
    x_slab = nc.dram_tensor("x_slab", [cfg.SLAB, H], F32, kind="ExternalInput")
    ident_in = nc.dram_tensor("ident", [128, 128], F16, kind="ExternalInput")
    dwT_in = nc.dram_tensor("dwT", [H, B], F16, kind="ExternalInput")
    c1_in = nc.dram_tensor("c1", [1, B], F16, kind="ExternalInput")
    pre_g_in = nc.dram_tensor("pre_g", [1, H], F16, kind="ExternalInput")
    pre_b_in = nc.dram_tensor("pre_b", [1, H], F16, kind="ExternalInput")
    d_out = nc.dram_tensor("d", [128, T * B], F16, kind="ExternalOutput")
    eta_out = nc.dram_tensor("eta", [128, T * H], F16, kind="ExternalOutput")

    with tile.TileContext(nc) as tc, ExitStack() as ctx:
        const = ctx.enter_context(tc.tile_pool(name="const", bufs=1))
        xin = ctx.enter_context(tc.tile_pool(name="xin", bufs=1))
        stat = ctx.enter_context(tc.tile_pool(name="stat", bufs=6))
        work = ctx.enter_context(tc.tile_pool(name="work", bufs=3))
        psT_p = ctx.enter_context(tc.tile_pool(name="psT", bufs=3, space="PSUM"))
        psD_p = ctx.enter_context(tc.tile_pool(name="psD", bufs=3, space="PSUM"))

        ident_t = const.tile([128, 128], F16)
        nc.scalar.dma_start(ident_t[:], ident_in[:])
        dwT_t = const.tile([H, B], F16)
        nc.scalar.dma_start(dwT_t[:], dwT_in[:])
        eps_t = const.tile([128, 1], F32)
        nc.vector.memset(eps_t[:], EPS)
        if cfg.use_bM:
            ones1 = const.tile([1, 128], F16)
            nc.vector.memset(ones1[:], 1.0)
            c1_t = const.tile([1, B], F16)
            nc.sync.dma_start(c1_t[:], c1_in[:])
        if cfg.use_pre_gb:
            pre_g_t = const.tile([1, H], F16)
            nc.sync.dma_start(pre_g_t[:], pre_g_in[:])
            pre_b_t = const.tile([1, H], F16)
            nc.sync.dma_start(pre_b_t[:], pre_b_in[:])
            ones_c = const.tile([1, 128], F16)
            nc.vector.memset(ones_c[:], 1.0)
            ps_g = psT_p.tile([128, H], F32, tag="pro", padded_shape=[128, 512])
            nc.tensor.matmul(ps_g[:], ones_c[:], pre_g_t[:], start=True, stop=True)
            gb_t = const.tile([128, H], F16)
            nc.scalar.activation(gb_t[:], ps_g[:], mybir.ActivationFunctionType.Copy)
            ps_b = psT_p.tile([128, H], F32, tag="pro", padded_shape=[128, 512])
            nc.tensor.matmul(ps_b[:], ones_c[:], pre_b_t[:], start=True, stop=True)
            bb_t = const.tile([128, H], F16)
            nc.scalar.activation(bb_t[:], ps_b[:], mybir.ActivationFunctionType.Copy)

        for _rep in range(cfg.reps):
            x_all = xin.tile([128, T, H], F32, tag="xall")
            x_src = x_slab.ap().rearrange("(t p) h -> p t h", p=128)
            HT = T // 2
            nc.sync.dma_start(x_all[:, 0:HT, :], x_src[:, 0:HT, :])
            nc.gpsimd.dma_start(x_all[:, HT:T, :], x_src[:, HT:T, :])
            eta_all = xin.tile([128, T, H], F16, tag="etaall")
            d_all = xin.tile([128, T, B], F16, tag="dall")
            live = {}

            def stage_a(t):
                xt = x_all[:, t, :]
                st6 = stat.tile([128, 6], F32, tag="st6", name="st6")
                nc.vector.bn_stats(st6[:], xt[:])
                mv = stat.tile([128, 2], F32, tag="mv", name="mv")
                nc.vector.bn_aggr(mv[:], st6[:])
                sd = stat.tile([128, 1], F32, tag="sd", name="sd")
                nc.scalar.activation(sd[:], mv[:, 1:2],
                                     mybir.ActivationFunctionType.Sqrt, bias=eps_t[:])
                rstd = stat.tile([128, 1], F32, tag="rstd", name="rstd")
                nc.vector.reciprocal(rstd[:], sd[:])
                live[t] = {"mv": mv, "rstd": rstd}

            def stage_b(t):
                mv, rstd = live[t]["mv"], live[t]["rstd"]
                eta = eta_all[:, t, :]
                nc.gpsimd.tensor_scalar(
                    eta, x_all[:, t, :], mv[:, 0:1], rstd[:],
                    mybir.AluOpType.subtract, mybir.AluOpType.mult,
                )
                psT = psT_p.tile([128, H], F16, tag="psT", padded_shape=[128, 1024], name="psT")
                nc.tensor.transpose(psT[:], eta, ident_t[:])
                etaT = work.tile([128, H], F16, tag="etaT", name="etaT")
                nc.scalar.activation(etaT[:], psT[:], mybir.ActivationFunctionType.Copy)
                live[t]["etaT"] = etaT

            def stage_c(t):
                etaT = live[t]["etaT"]
                psD = psD_p.tile([128, B], F32, tag="psD", padded_shape=[128, 512], name="psD")
                nc.tensor.matmul(psD[:], etaT[:], dwT_t[:], start=True,
                                 stop=not cfg.use_bM)
                if cfg.use_bM:
                    nc.tensor.matmul(psD[:], ones1[:], c1_t[:], start=False, stop=True)
                nc.vector.tensor_copy(d_all[:, t, :], psD[:])
                if cfg.use_pre_gb:
                    eta = eta_all[:, t, :]
                    nc.vector.tensor_tensor(eta, eta, gb_t[:], mybir.AluOpType.mult)
                    nc.vector.tensor_tensor(eta, eta, bb_t[:], mybir.AluOpType.add)
                del live[t]

            for tt in range(T + 2):
                if tt < T:
                    stage_a(tt)
                if 1 <= tt <= T:
                    stage_b(tt - 1)
                if tt >= 2:
                    stage_c(tt - 2)
            eta_dst = eta_out.ap().rearrange("p (t h) -> p t h", t=T)
            nc.sync.dma_start(eta_dst[:, 0:HT, :], eta_all[:, 0:HT, :])
            nc.sync.dma_start(eta_dst[:, HT:T, :], eta_all[:, HT:T, :])
            nc.sync.dma_start(d_out.ap().rearrange("p (t b) -> p t b", t=T), d_all[:])

    nc.compile()
    return nc


def build_k2(cfg: Cfg):
    nc = bacc.Bacc("TRN2", target_bir_lowering=False, debug=False, num_devices=cfg.NCORES)
    H, B, T, G = cfg.H, cfg.B, cfg.T, cfg.G
    ngs = cfg.ngs
    NGmax = max(ngs)
    tot_ng = sum(ngs)
    # idx cols per tile: num_idxs_t/16 = ngs[t]*G*128/16 = ngs[t]*G*8
    tot_cols = tot_ng * G * 8

    table_in = nc.dram_tensor("table", [cfg.N + 1, 64], I32, kind="ExternalInput")
    eta_in = nc.dram_tensor("eta", [128, T * H], F16, kind="ExternalInput")
    idx_in = nc.dram_tensor("idx_plane", [128, tot_cols], I16, kind="ExternalInput")
    R_in = nc.dram_tensor("R_plane", [128, tot_ng], F32, kind="ExternalInput")
    iota_in = nc.dram_tensor("iota", [128, 128], F16, kind="ExternalInput")
    ident_in = nc.dram_tensor("ident", [128, 128], F16, kind="ExternalInput")
    upwT_in = nc.dram_tensor("upwT4", [128, H], F16, kind="ExternalInput")
    c_in = nc.dram_tensor("c_t", [1, H], F16, kind="ExternalInput")
    post_g_in = nc.dram_tensor("post_g", [1, H], F16, kind="ExternalInput")
    post_b_in = nc.dram_tensor("post_b", [1, H], F16, kind="ExternalInput")
    y_out = nc.dram_tensor("y", [128, T * H], F16, kind="ExternalOutput")

    with tile.TileContext(nc) as tc, ExitStack() as ctx:
        const = ctx.enter_context(tc.tile_pool(name="const", bufs=1))
        stat = ctx.enter_context(tc.tile_pool(name="stat", bufs=8))
        ohp = ctx.enter_context(tc.tile_pool(name="oh", bufs=24))
        gathp = ctx.enter_context(tc.tile_pool(name="gath", bufs=3))
        outp = ctx.enter_context(tc.tile_pool(name="outp", bufs=6))
        psAB = ctx.enter_context(tc.tile_pool(name="psAB", bufs=3, space="PSUM"))
        psTp = ctx.enter_context(tc.tile_pool(name="psT", bufs=2, space="PSUM"))
        psZp = ctx.enter_context(tc.tile_pool(name="psZ", bufs=3, space="PSUM"))

        idxp_t = const.tile([128, tot_cols], I16)
        HC = tot_cols // 2
        nc.sync.dma_start(idxp_t[:, 0:HC], idx_in[:, 0:HC])
        Rp_t = const.tile([128, tot_ng], F32)
        nc.scalar.dma_start(Rp_t[:], R_in[:])
        iota_t = const.tile([128, 128], F16)
        nc.scalar.dma_start(iota_t[:], iota_in[:])
        ident_t = const.tile([128, 128], F16)
        nc.scalar.dma_start(ident_t[:], ident_in[:])
        upwT_t = const.tile([128, H], F16)
        nc.scalar.dma_start(upwT_t[:], upwT_in[:])
        nc.sync.dma_start(idxp_t[:, HC:tot_cols], idx_in[:, HC:tot_cols])
        eps_t = const.tile([128, 1], F32)
        nc.vector.memset(eps_t[:], EPS)
        if cfg.use_c:
            ones1 = const.tile([1, 128], F16)
            nc.vector.memset(ones1[:], 1.0)
            c_t = const.tile([1, H], F16)
            nc.sync.dma_start(c_t[:], c_in[:])
        if cfg.use_post_gb:
            ones_c = const.tile([1, 128], F16)
            nc.vector.memset(ones_c[:], 1.0)
            post_g_t = const.tile([1, H], F16)
            nc.sync.dma_start(post_g_t[:], post_g_in[:])
            post_b_t = const.tile([1, H], F16)
            nc.sync.dma_start(post_b_t[:], post_b_in[:])
            ps_g = psZp.tile([128, H], F32, tag="psZ", padded_shape=[128, 512])
            nc.tensor.matmul(ps_g[:], ones_c[:], post_g_t[:], start=True, stop=True)
            postg_b = const.tile([128, H], F16)
            nc.scalar.activation(postg_b[:], ps_g[:], mybir.ActivationFunctionType.Copy)
            ps_b = psZp.tile([128, H], F32, tag="psZ", padded_shape=[128, 512])
            nc.tensor.matmul(ps_b[:], ones_c[:], post_b_t[:], start=True, stop=True)
            postb_b = const.tile([128, H], F16)
            nc.scalar.activation(postb_b[:], ps_b[:], mybir.ActivationFunctionType.Copy)

        for _rep in range(cfg.reps):
            eta_all = const.tile([128, T, H], F16)
            nc.scalar.dma_start(eta_all[:], eta_in.ap().rearrange("p (t h) -> p t h", t=T))
            y_all = const.tile([128, T, H], F16)

            colb = [sum(ngs[:i]) for i in range(T)]
            icolb = [sum(ngs[:i]) * G * 8 for i in range(T)]
            live = {}  # t -> dict of tiles crossing stage boundaries

            def stage_a(t):
                NG = ngs[t]
                n_idx = NG * G * 128
                gath = gathp.tile([128, NGmax * G, 32], I32, tag="gath", name="gath")
                _raw_dma_gather(
                    nc.gpsimd,
                    gath[:, 0 : NG * G, :],
                    table_in.ap()[:, 0:32],
                    idxp_t[:, icolb[t] : icolb[t] + n_idx // 16],
                    num_idxs=n_idx,
                    num_idxs_reg=n_idx,
                    elem_size=32,
                    elem_step=64,
                )
                ohs = []
                for g in range(NG):
                    oh = ohp.tile([128, 128], F16, tag="oh", name="oh")
                    nc.vector.tensor_scalar(
                        oh[:], iota_t[:], Rp_t[:, colb[t] + g : colb[t] + g + 1],
                        None, mybir.AluOpType.is_equal,
                    )
                    ohs.append(oh)
                live[t] = {"gath": gath, "ohs": ohs}

            def stage_b(t):
                NG = ngs[t]
                gath = live[t]["gath"]
                ohs = live[t]["ohs"]
                gf = gath[:].bitcast(F16)  # [128, NGmax*G, 64]; cols 0:32 = d
                psA = psAB.tile([128, 128], F32, tag="psA", padded_shape=[128, 512], name="psA")
                for g in range(NG):
                    # psA[m, je*32+c] += sum_k oh[k, m] * d[dst(g,k,je)][c]
                    nc.tensor.matmul(psA[:], ohs[g][:], gf[:, g * G : g * G + G, 0:32],
                                     start=(g == 0), stop=(g == NG - 1))
                sbA = outp.tile([128, 128], F16, tag="sbA", name="sbA")
                nc.scalar.activation(sbA[:], psA[:], mybir.ActivationFunctionType.Copy)
                psT = psTp.tile([128, 128], F16, tag="psT", padded_shape=[128, 1024], name="psT")
                nc.tensor.transpose(psT[:], sbA[:], ident_t[:])
                sbT = outp.tile([128, 128], F16, tag="sbT", name="sbT")
                nc.scalar.activation(sbT[:], psT[:], mybir.ActivationFunctionType.Copy)
                psZ = psZp.tile([128, H], F32, tag="psZ", padded_shape=[128, 512], name="psZ")
                # z2[m, h] = sum_{je,c} aggT[32*je+c, m] * upwT4[32*je+c, h]
                nc.tensor.matmul(psZ[:], sbT[:], upwT_t[:], start=True,
                                 stop=not cfg.use_c)
                if cfg.use_c:
                    nc.tensor.matmul(psZ[:], ones1[:], c_t[:], start=False, stop=True)
                live[t]["psZ"] = psZ

            def stage_c(t):
                psZ = live[t]["psZ"]
                v = outp.tile([128, H], F16, tag="v", name="v")
                nc.scalar.activation(v[:], psZ[:], mybir.ActivationFunctionType.Relu)
                v2 = outp.tile([128, H], F16, tag="v2", name="v2")
                nc.gpsimd.tensor_tensor(v2[:], v[:], eta_all[:, t, :], mybir.AluOpType.add)
                st6 = stat.tile([128, 6], F32, tag="st6", name="st6")
                nc.vector.bn_stats(st6[:], v2[:])
                mv = stat.tile([128, 2], F32, tag="mv", name="mv")
                nc.vector.bn_aggr(mv[:], st6[:])
                sd = stat.tile([128, 1], F32, tag="sd", name="sd")
                nc.scalar.activation(sd[:], mv[:, 1:2],
                                     mybir.ActivationFunctionType.Sqrt, bias=eps_t[:])
                rstd = stat.tile([128, 1], F32, tag="rstd", name="rstd")
                nc.vector.reciprocal(rstd[:], sd[:])
                yt = y_all[:, t, :]
                nc.vector.tensor_scalar(
                    yt, v2[:], mv[:, 0:1], rstd[:],
                    mybir.AluOpType.subtract, mybir.AluOpType.mult,
                )
                if cfg.use_post_gb:
                    nc.vector.tensor_tensor(yt, yt, postg_b[:], mybir.AluOpType.mult)
                    nc.vector.tensor_tensor(yt, yt, postb_b[:], mybir.AluOpType.add)
                del live[t]

            for tt in range(T + 2):
                if tt < T:
                    stage_a(tt)
                if 1 <= tt <= T:
                    stage_b(tt - 1)
                if tt >= 2:
                    stage_c(tt - 2)
            y_dst = y_out.ap().rearrange("p (t h) -> p t h", t=T)
            HT = T // 2
            nc.sync.dma_start(y_dst[:, 0:HT, :], y_all[:, 0:HT, :])
            nc.sync.dma_start(y_dst[:, HT:T, :], y_all[:, HT:T, :])

    nc.compile()
    return nc


# ---------------------------------------------------------------------------
# host-side prep
# ---------------------------------------------------------------------------


def prep_inputs(x, edge_index, down_w, down_b, up_w, up_b, pre_g, pre_b, post_g,
                post_b, cfg=None):
    N = x.shape[1]
    H = x.shape[2]
    B = down_w.shape[0]
    src = np.asarray(edge_index[0], dtype=np.int64)
    dst = np.asarray(edge_index[1], dtype=np.int64)

    if cfg is None:
        cfg = Cfg(N=N, H=H, B=B)
    G = cfg.G
    T = cfg.T
    n_tiles_total = N // 128

    order = np.argsort(src, kind="stable")
    src_s = src[order]
    dst_s = dst[order]
    tile_of = (src_s >> 7).astype(np.int64)
    row = (src_s & 127).astype(np.int64)
    cnt = np.bincount(src_s, minlength=N)
    row_start = np.concatenate([[0], np.cumsum(cnt)])
    pos_in_row = np.arange(len(src_s)) - row_start[src_s]
    claim_in_row = pos_in_row // G
    je = pos_in_row % G
    cpr = (cnt + G - 1) // G  # claims per src row
    cpr_t = cpr.reshape(n_tiles_total, 128)
    claim_base = np.cumsum(cpr_t, axis=1) - cpr_t  # within-tile claim offset per row
    claims_per_tile = cpr_t.sum(axis=1)
    claim_idx = claim_base[tile_of, row] + claim_in_row
    g_of = claim_idx // 128
    k_of = claim_idx % 128
    slot_of = (g_of * G + je) * 128 + k_of

    ng_per_tile = np.maximum(1, -(-claims_per_tile // 128)).reshape(cfg.NCORES, T)
    ngs = tuple(int(v) for v in ng_per_tile.max(axis=0))
    cfg.ngs = ngs
    cfg.use_bM = bool(np.any(pre_b != 0))
    cfg.use_c = bool(np.any(down_b != 0) or np.any(up_b != 0))
    cfg.use_pre_gb = bool(np.any(pre_g != 1) or np.any(pre_b != 0))
    cfg.use_post_gb = bool(np.any(post_g != 1) or np.any(post_b != 0))

    bounds = np.searchsorted(tile_of, np.arange(n_tiles_total + 1))
    tot_ng = sum(ngs)

    ident = np.eye(128, dtype=np.float16)
    iota = np.tile(np.arange(128, dtype=np.float16), (128, 1))
    dw_eff = (np.asarray(down_w, np.float32) * np.asarray(pre_g, np.float32)[None, :])
    dwT = np.ascontiguousarray(dw_eff.T).astype(np.float16)  # [H, B]
    c1 = (np.asarray(pre_b, np.float32) @ dw_eff.T).reshape(1, B).astype(np.float16)
    upwT4 = np.tile(np.asarray(up_w, np.float32).T, (128 // B, 1)).astype(np.float16)  # [128, H]
    c_t = (np.asarray(down_b, np.float32) @ np.asarray(up_w, np.float32).T
           + np.asarray(up_b, np.float32)).reshape(1, H).astype(np.float16)

    k1_maps, k2_maps = [], []
    for c in range(cfg.NCORES):
        fl_parts = []
        Rp = np.zeros((128, tot_ng), np.float32)
        col0 = 0
        for t in range(T):
            gt = c * T + t
            a, b = bounds[gt], bounds[gt + 1]
            n_slots = ngs[t] * G * 128
            fl = np.full(n_slots, N, dtype=np.int64)
            fl[slot_of[a:b]] = dst_s[a:b]
            fl_parts.append(np.tile(fl.reshape(-1, 16).T, (8, 1)))
            ncl = claims_per_tile[gt]
            rr = np.repeat(np.arange(128), cpr_t[gt])  # src row per claim
            ci = np.arange(ncl)
            Rp[ci % 128, col0 + ci // 128] = rr
            col0 += ngs[t]
        idx_plane = np.ascontiguousarray(
            np.concatenate(fl_parts, axis=1).astype(np.int16))

        k1_maps.append({
            "x_slab": np.ascontiguousarray(x[0, c * cfg.SLAB : (c + 1) * cfg.SLAB, :], dtype=np.float32),
            "ident": ident,
            "dwT": dwT,
            "c1": c1,
            "pre_g": np.asarray(pre_g, np.float16).reshape(1, H),
            "pre_b": np.asarray(pre_b, np.float16).reshape(1, H),
        })
        k2_maps.append({
            "idx_plane": idx_plane,
            "R_plane": np.ascontiguousarray(Rp),
            "iota": iota,
            "ident": ident,
            "upwT4": upwT4,
            "c_t": c_t,
            "post_g": np.asarray(post_g, np.float16).reshape(1, H),
            "post_b": np.asarray(post_b, np.float16).reshape(1, H),
        })
    return cfg, k1_maps, k2_maps


def table_from_d(cfg, d_list):
    """d_list: per-core [128, T*B] f16 -> int32 table [N+1, 64] (256B rows)."""
    B = cfg.B
    parts = []
    for arr in d_list:
        a = np.asarray(arr).astype(np.float16).reshape(128, cfg.T, B)
        parts.append(np.ascontiguousarray(a.transpose(1, 0, 2)).reshape(cfg.SLAB, B))
    d_full = np.concatenate(parts, axis=0)
    tabf = np.zeros((cfg.N + 1, 128), np.float16)
    tabf[: cfg.N, :B] = d_full
    return tabf.view(np.int32)


def y_from_outs(cfg, y_list):
    """per-core [128, T*H] f16 -> [1, N, H] f32."""
    H = cfg.H
    parts = []
    for arr in y_list:
        a = np.asarray(arr).astype(np.float32).reshape(128, cfg.T, H)
        parts.append(np.ascontiguousarray(a.transpose(1, 0, 2)).reshape(cfg.SLAB, H))
    return np.concatenate(parts, axis=0)[None]


# ---------------------------------------------------------------------------
# main entry
# ---------------------------------------------------------------------------

_CACHE = {}


def _run_spmd(nc, maps, cores):
    # int64 table input requires x64 through the bass2jax/PJRT path
    try:
        import jax
        jax.config.update("jax_enable_x64", True)
    except Exception:
        pass
    from concourse.bass_utils import run_bass_kernel_spmd

    last_err = None
    for _attempt in range(3):
        try:
            return run_bass_kernel_spmd(nc, maps, cores).results
        except Exception as e:  # transient device/transport errors
            last_err = e
            import time as _time
            _time.sleep(2.0)
    raise last_err


def kernel(x, edge_index, down_w, down_b, up_w, up_b, pre_g, pre_b, post_g, post_b):
    import numpy as _np

    inputs = dict(x=_np.asarray(x), edge_index=_np.asarray(edge_index),
                  down_w=_np.asarray(down_w), down_b=_np.asarray(down_b),
                  up_w=_np.asarray(up_w), up_b=_np.asarray(up_b),
                  pre_g=_np.asarray(pre_g), pre_b=_np.asarray(pre_b),
                  post_g=_np.asarray(post_g), post_b=_np.asarray(post_b))
    cfg, k1_maps, k2_maps = prep_inputs(**inputs)
    key = (cfg.N, cfg.H, cfg.B, cfg.G, cfg.ngs, cfg.use_bM, cfg.use_c,
           cfg.use_pre_gb, cfg.use_post_gb)
    if key not in _CACHE:
        _CACHE[key] = (build_k1(cfg), build_k2(cfg))
    nc1, nc2 = _CACHE[key]
    cores = list(range(cfg.NCORES))
    r1 = _run_spmd(nc1, k1_maps, cores)
    table = table_from_d(cfg, [r1[c]["d"] for c in range(cfg.NCORES)])
    for c in range(cfg.NCORES):
        k2_maps[c]["table"] = table
        k2_maps[c]["eta"] = r1[c]["eta"]
    r2 = _run_spmd(nc2, k2_maps, cores)
    return y_from_outs(cfg, [r2[c]["y"] for c in range(cfg.NCORES)]).astype(_np.float32)
